# revision 48
# baseline (speedup 1.0000x reference)
"""Trainium2 Bass kernel for CausalMessagePassing (B=8, N=2048, D=256, H=4).

Strategy: data-parallel across 8 NeuronCores, one graph per core.
Per-core dataflow (v2 "row-major ctx" redesign):
  x -> x^T (PE transpose, bf16 identity); q^T,k^T col-major f32r; v
  row-major bf16 with a ones column per head.
  scores^T[j,i] = k_h^T.T @ q_h^T per head in f32r over the causal
  triangle only (mask input never DMA'd; tril structure exploited).
  e = exp(scores/8) -> bf16, split across engines: ACT (exact) plus
  Pool/DVE tiles using a Schraudolph bf16 approximation (int16 bitcast);
  softmax normalization cancels its systematic component.
  Diagonal-tile causal mask = multiply with a constant upper-tri bf16
  mask on DVE (4x mode).
  ctx row-major per i-block: ctx[i, h*65+c] accumulates e-chunk^T @
  [v_h | 1] with all 4 heads sharing one PSUM bank (K=1 zero-row opener
  matmul establishes the accumulation group + write ordering).
  Norm: DVE reciprocal of the 4 sums columns + broadcast multiply
  during PSUM eviction -> ctx_n bf16; PE-transpose (bf16) back to
  col-major ectx for the Wo spine.  messages^T = Wo.T @ ectx (+bo).
  u computed ROW-major: u[i,:] = relu(sum_c chunk_c(x^T,m^T).T @ Wu_c
  + bu) with bu folded in via a K=1 opener matmul; direct DMA out, no
  output transposes.
  Extras: PE HAM warm-up + ACT exp-table preload during the input DMA
  window; software-pipelined exp->ctx; projection/tail work interleaved
  into the attention stream to fill PE stall gaps.
"""
import sys

sys.path.insert(0, "/opt/trn_rl_repo")

import numpy as np

import concourse.bass as bass  # noqa: F401
import concourse.mybir as mybir
import concourse.tile as tile
from concourse import bacc
from concourse.masks import make_identity

B, N, DM, H = 8, 2048, 256, 4
HD = DM // H  # 64
NB = N // 128  # 16 i/j blocks
IT = N // 512  # 4 quarters
F32 = mybir.dt.float32
F32R = mybir.dt.float32r
BF16 = mybir.dt.bfloat16
I16 = mybir.dt.int16

# Schraudolph bf16 exp: i16 = trunc(x * SCHR_S + SCHR_B); bitcast -> bf16
SCHR_S = 128.0 / np.log(2.0)
SCHR_B = 127.0 * 128.0 - 5.1
SCALE = 0.125  # 1/sqrt(HD)

# exp-tile engine assignment pattern (A=ACT exact, P=Pool, D=DVE approx)
EXP_PATTERN = "AAAAAPPD"


def build_program():
    nc = bacc.Bacc("TRN2", target_bir_lowering=False, debug=False)
    x_d = nc.dram_tensor("x", [N, DM], F32, kind="ExternalInput").ap()
    wq_d = nc.dram_tensor("wq", [DM, DM], F32, kind="ExternalInput").ap()
    wk_d = nc.dram_tensor("wk", [DM, DM], F32, kind="ExternalInput").ap()
    wv_d = nc.dram_tensor("wv", [DM, DM], F32, kind="ExternalInput").ap()
    wo_d = nc.dram_tensor("wo", [DM, DM], F32, kind="ExternalInput").ap()
    wu_d = nc.dram_tensor("wu", [2 * DM, DM], F32, kind="ExternalInput").ap()
    bq_d = nc.dram_tensor("bq", [DM], F32, kind="ExternalInput").ap()
    bk_d = nc.dram_tensor("bk", [DM], F32, kind="ExternalInput").ap()
    bv_d = nc.dram_tensor("bv", [DM], F32, kind="ExternalInput").ap()
    bo_d = nc.dram_tensor("bo", [DM], F32, kind="ExternalInput").ap()
    bu_d = nc.dram_tensor("bu", [DM], F32, kind="ExternalInput").ap()
    out_d = nc.dram_tensor("out", [N, DM], F32, kind="ExternalOutput").ap()

    def r(ap):
        return ap.bitcast(F32R)

    with tile.TileContext(nc) as tc:
        with (
            tc.tile_pool(name="const", bufs=1) as cpool,
            tc.tile_pool(name="big", bufs=1) as bpool,
            tc.tile_pool(name="work", bufs=3) as wpool,
            tc.tile_pool(name="mm", bufs=2, space="PSUM") as mmp,
            tc.tile_pool(name="sc", bufs=3, space="PSUM") as scp,
            tc.tile_pool(name="ctxp", bufs=2, space="PSUM") as ctxp,
            tc.tile_pool(name="sump", bufs=1, space="PSUM") as sump,
        ):
            # ---- constants / weights (batched DMAs) ----
            ident_f = cpool.tile([128, 128], F32, tag="identf")
            make_identity(nc, ident_f[:])
            ident = cpool.tile([128, 128], F32R, tag="ident")
            nc.vector.tensor_copy(ident[:], ident_f[:])
            ident_b = cpool.tile([128, 128], BF16, tag="identb")
            nc.vector.tensor_copy(ident_b[:], ident_f[:])
            # PE HAM warm-up during the input-DMA window; ACT exp-table
            # preload off the critical path.
            warm = scp.tile([128, 512], F32R, tag="sc", name="warm")
            for _ in range(12):
                nc.tensor.transpose(warm[0:128, 0:128], ident[:], ident[:])
            wexp = cpool.tile([1, 8], F32, tag="wexp")
            nc.scalar.activation(
                wexp[:], ident_f[0:1, 0:8], mybir.ActivationFunctionType.Exp
            )
            # each W loaded as one DMA: [128, 2*DM], chunk c at cols [c*DM,..)
            wq_a = cpool.tile([128, 2 * DM], F32R, tag="wqa")
            wk_a = cpool.tile([128, 2 * DM], F32R, tag="wka")
            wv_a = cpool.tile([128, 2 * DM], F32R, tag="wva")
            wo_a = cpool.tile([128, 2 * DM], F32R, tag="woa")
            wu_a = cpool.tile([128, 4 * DM], F32R, tag="wua")

            def dma_w(t_sb, t_d):
                nc.sync.dma_start(
                    t_sb[:].rearrange("p (c d) -> p c d", d=DM),
                    r(t_d.rearrange("(c p) d -> p c d", p=128)),
                )

            stage = cpool.tile([128, NB * DM], F32R, tag="stage")
            xs_all = stage
            x_r = r(x_d.rearrange("(t p) d -> p t d", p=128))

            def dma_x(g):
                nc.sync.dma_start(
                    xs_all[:, g * 2 * DM:(g + 1) * 2 * DM].rearrange(
                        "p (t d) -> p t d", d=DM
                    ),
                    x_r[:, g * 2:(g + 1) * 2, :],
                )

            dma_x(0)
            dma_x(1)
            dma_w(wq_a, wq_d)
            dma_w(wk_a, wk_d)
            dma_w(wv_a, wv_d)
            wq_sb = [wq_a[:, c * DM:(c + 1) * DM] for c in range(2)]
            wk_sb = [wk_a[:, c * DM:(c + 1) * DM] for c in range(2)]
            wv_sb = [wv_a[:, c * DM:(c + 1) * DM] for c in range(2)]
            wu_sb = [wu_a[:, c * DM:(c + 1) * DM] for c in range(4)]
            bq_a = cpool.tile([128, 2], F32, tag="bqa")
            bk_a = cpool.tile([128, 2], F32, tag="bka")
            bo_a = cpool.tile([128, 2], F32, tag="boa")
            for t_sb, t_d in ((bq_a, bq_d), (bk_a, bk_d), (bo_a, bo_d)):
                nc.sync.dma_start(t_sb[:], t_d.rearrange("(c p) -> p c", p=128))
            bq_c = [bq_a[:, b:b + 1] for b in range(2)]
            bk_c = [bk_a[:, b:b + 1] for b in range(2)]
            bo_c = [bo_a[:, b:b + 1] for b in range(2)]
            # bu as a row [1, 256] for the K=1 bias-opener matmul
            bu_row = cpool.tile([1, DM], F32, tag="burow")
            nc.sync.dma_start(bu_row[:], bu_d.rearrange("(b a) -> b a", b=1))
            # bv broadcast tile [128, 256] (f32; used by DVE add)
            bv_row = cpool.tile([1, DM], F32, tag="bvrow")
            nc.sync.dma_start(bv_row[:], bv_d.rearrange("(b a) -> b a", b=1))
            ones1 = cpool.tile([1, 128], F32, tag="ones1")
            nc.gpsimd.memset(ones1[:], 1.0)
            bv_bc = cpool.tile([128, DM], F32, tag="bvbc")
            pt0 = mmp.tile([128, DM], F32, tag="mm")
            nc.tensor.matmul(pt0[:], ones1[:], bv_row[:], start=True, stop=True)
            nc.vector.tensor_copy(bv_bc[:], pt0[:])
            zrow = cpool.tile([1, 512], F32, tag="zrow")
            nc.gpsimd.memset(zrow[:], 0.0)
            ones_col4 = cpool.tile([128, 4], BF16, tag="onescol4")
            nc.gpsimd.memset(ones_col4[:], 1.0)
            # constant upper-tri bf16 mask (keep col >= row); built lazily so
            # it doesn't delay the startup Pool queue
            ut_b = cpool.tile([128, 128], BF16, tag="utb")

            def unit_ut():
                nc.gpsimd.memset(ut_b[:], 1.0)
                nc.gpsimd.affine_select(
                    ut_b[:], ut_b[:],
                    pattern=[[1, 128]],
                    compare_op=mybir.AluOpType.is_ge,
                    fill=0.0,
                    base=0,
                    channel_multiplier=-1,
                )
            # ---- remaining weights, then rest of x (x blocks 4+ are not
            # needed until the second quarter's transposes) ----
            dma_w(wo_a, wo_d)
            dma_w(wu_a, wu_d)
            for g in range(2, 8):
                dma_x(g)
            # Wo in bf16 so the m spine can use narrow (128-col) moving
            # tiles; conversion emitted lazily (as a quarter-0 filler) so it
            # doesn't head-block the DVE queue while the Wo DMA lands.
            wo_b = cpool.tile([128, 2 * DM], BF16, tag="wob")
            wo_sb = [wo_b[:, c * DM:(c + 1) * DM] for c in range(2)]

            def unit_wo_conv():
                nc.vector.tensor_copy(wo_b[:], wo_a[:])

            xT = [bpool.tile([128, N], F32R, tag=f"xT{c}", name=f"xT{c}") for c in range(2)]
            qT = [bpool.tile([128, N], F32R, tag=f"qT{b}", name=f"qT{b}") for b in range(2)]
            kT = [bpool.tile([128, N], F32R, tag=f"kT{b}", name=f"kT{b}") for b in range(2)]
            ectx = [bpool.tile([128, N], BF16, tag=f"ectx{b}", name=f"ectx{b}") for b in range(2)]
            m_sb = [bpool.tile([128, N], F32R, tag=f"m{b}", name=f"m{b}") for b in range(2)]
            # v_sb[jb]: [128, 4*65] bf16; head h data at 65h..65h+63, ones at 65h+64
            v_sb = [bpool.tile([128, 4 * 65], BF16, tag=f"v{jb}", name=f"v{jb}") for jb in range(NB)]
            ostage = stage
            out_r = r(out_d.rearrange("(t p) d -> p t d", p=128))

            # ---------- filler units (projection / tail work) ----------
            def unit_xT(ib, alternate=False):
                for c in range(2):
                    tp = scp.tile([128, 128], F32R, tag="sc", name="tp")
                    nc.tensor.transpose(
                        tp[:], xs_all[:, ib * DM + c * 128:ib * DM + (c + 1) * 128],
                        ident_b[:],
                    )
                    eng = (
                        nc.vector if alternate and (2 * ib + c) % 2 == 0
                        else nc.gpsimd
                    )
                    eng.tensor_copy(xT[c][:, ib * 128:(ib + 1) * 128], tp[:])

            def unit_qk(it, which, blk):
                w_sb, b_c, dstT = (
                    (wq_sb, bq_c, qT) if which == 0 else (wk_sb, bk_c, kT)
                )
                pt = mmp.tile([128, 512], F32, tag="mm", name="qkpt")
                for c in range(2):
                    nc.tensor.matmul(
                        pt[:],
                        w_sb[c][:, blk * 128:(blk + 1) * 128],
                        xT[c][:, it * 512:(it + 1) * 512],
                        start=(c == 0),
                        stop=(c == 1),
                    )
                nc.vector.tensor_scalar_add(
                    dstT[blk][:, it * 512:(it + 1) * 512], pt[:], b_c[blk][:]
                )

            def unit_v(jb):
                v4 = v_sb[jb][:].rearrange("p (h e) -> p h e", e=65)
                nc.vector.tensor_copy(
                    v4[:, :, 64:65],
                    ones_col4[:].rearrange("p (h e) -> p h e", e=1),
                )
                pt = mmp.tile([128, DM], F32, tag="mm", name="vpt")
                for c in range(2):
                    nc.tensor.matmul(
                        pt[:],
                        xT[c][:, jb * 128:(jb + 1) * 128],
                        wv_sb[c][:],
                        start=(c == 0),
                        stop=(c == 1),
                    )
                nc.vector.tensor_tensor(
                    v4[:, :, 0:64],
                    pt[:].rearrange("p (h e) -> p h e", e=64),
                    bv_bc[:].rearrange("p (h e) -> p h e", e=64),
                    op=mybir.AluOpType.add,
                )

            def unit_ctxT(ib, ctxn):
                for c in range(2):
                    tp = scp.tile([128, 128], BF16, tag="sc", name="tpc")
                    nc.tensor.transpose(
                        tp[:], ctxn[:, c * 128:(c + 1) * 128], ident_b[:]
                    )
                    nc.gpsimd.tensor_copy(ectx[c][:, ib * 128:(ib + 1) * 128], tp[:])

            def unit_m(ib, blk):
                # messages^T for one i-block (128 cols, bf16 moving)
                isl = slice(ib * 128, (ib + 1) * 128)
                pt = mmp.tile([128, 128], F32, tag="mm", name="mpt")
                for c in range(2):
                    nc.tensor.matmul(
                        pt[:],
                        wo_sb[c][:, blk * 128:(blk + 1) * 128],
                        ectx[c][:, isl],
                        start=(c == 0),
                        stop=(c == 1),
                    )
                nc.vector.tensor_scalar_add(m_sb[blk][:, isl], pt[:], bo_c[blk][:])

            def unit_u(ib):
                up = mmp.tile([128, DM], F32, tag="mm", name="up")
                # K=1 bias opener: writes bu to every row, claims the group
                nc.tensor.matmul(
                    up[:], r(ones1[:]), r(bu_row[:]), start=True, stop=False,
                    skip_group_check=True,
                )
                for c in range(4):
                    lhs = xT[c] if c < 2 else m_sb[c - 2]
                    nc.tensor.matmul(
                        up[:],
                        lhs[:, ib * 128:(ib + 1) * 128],
                        wu_sb[c],
                        start=False,
                        stop=(c == 3),
                        skip_group_check=True,
                    )
                nc.vector.tensor_scalar_max(
                    ostage[:, ib * DM:(ib + 1) * DM], up[:], 0.0
                )
                nc.sync.dma_start(
                    out_r[:, ib:ib + 1, :],
                    ostage[:, ib * DM:(ib + 1) * DM].rearrange(
                        "p (t d) -> p t d", d=DM
                    ),
                )

            # ---------- attention ----------
            exp_idx = [0]

            def emit_exp(e, sc_t, skip, w):
                kind = EXP_PATTERN[exp_idx[0] % len(EXP_PATTERN)]
                exp_idx[0] += 1
                if kind == "A":
                    nc.scalar.activation(
                        e[:, skip:w], sc_t[:, skip:w],
                        mybir.ActivationFunctionType.Exp,
                        scale=SCALE,
                    )
                else:
                    eng = nc.gpsimd if kind == "P" else nc.vector
                    eng.tensor_scalar(
                        e[:, skip:w].bitcast(I16), sc_t[:, skip:w],
                        float(SCHR_S * SCALE), float(SCHR_B),
                        op0=mybir.AluOpType.mult, op1=mybir.AluOpType.add,
                    )

            mask_idx = [0]

            # ---------- flat attention pipeline across all quarters ----------
            g_i = [0]      # global group counter
            sched = []     # (due_group, fn) deferred tail stages
            pend = []      # scores->ctx software pipeline entries

            def enq(delay, fn):
                sched.append((g_i[0] + delay, fn))

            def run_due():
                for item in list(sched):
                    if item[0] <= g_i[0]:
                        sched.remove(item)
                        item[1]()

            def tail_norm(it, ib, ctx_t, sums):
                ib4 = ib % 4
                rsb = wpool.tile([128, 4], F32, tag="rsb", bufs=2, name="rsb")
                nc.vector.reciprocal(
                    rsb[:].rearrange("p (h e) -> p h e", e=1),
                    sums[:].rearrange("p (h e) -> p h e", e=1)[:, 4 * ib4:4 * ib4 + 4],
                )
                c4 = ctx_t[ib4 // 2][:, 256 * (ib4 % 2):256 * (ib4 % 2) + 256]
                ctxn = wpool.tile([128, DM], BF16, tag="ctxn", bufs=4, name="ctxn")
                in0 = c4.rearrange("p (h e) -> p h e", e=64)
                in1 = rsb[:].rearrange("p (h e) -> p h e", e=1)
                in0b, in1b = bass.broadcast_tensor_aps(in0, in1)
                nc.vector.tensor_tensor(
                    ctxn[:].rearrange("p (h e) -> p h e", e=64),
                    in0b, in1b, op=mybir.AluOpType.mult,
                )
                enq(2, lambda: tail_ctxT(ib, ctxn))

            def tail_ctxT(ib, ctxn):
                unit_ctxT(ib, ctxn)
                enq(2, lambda: tail_m(ib))

            def tail_m(ib):
                for blk in range(2):
                    unit_m(ib, blk)
                enq(2, lambda: tail_u(ib))

            def tail_u(ib):
                unit_u(ib)

            def emit_ctx(ent):
                jb, h, e, cst, it, st = ent
                ctx_t, sums = st["ctx"], st["sums"]
                if not st["opened"]:
                    # deferred group-openers: must land after the previous
                    # quarter's norm reads of the same PSUM slots
                    st["opened"] = True
                    for i in range(2):
                        nc.tensor.matmul(
                            ctx_t[i][:], r(ones1[:]), r(zrow[:]), start=True,
                            stop=False, skip_group_check=True,
                        )
                    nc.tensor.matmul(
                        sums[:], r(ones1[:]), r(zrow[0:1, 0:16]), start=True,
                        stop=False, skip_group_check=True,
                    )
                for ib in range(max(4 * it, jb), 4 * it + 4):
                    ib4 = ib % 4
                    eoff = 128 * ib4 - (cst - 512 * it)
                    nc.tensor.matmul(
                        ctx_t[ib4 // 2][:, 256 * (ib4 % 2) + 64 * h:
                                        256 * (ib4 % 2) + 64 * h + 64],
                        e[:, eoff:eoff + 128],
                        v_sb[jb][:, 65 * h:65 * h + 64],
                        start=False,
                        stop=(jb == ib),
                        skip_group_check=True,
                    )
                    nc.tensor.matmul(
                        sums[:, 4 * ib4 + h:4 * ib4 + h + 1],
                        e[:, eoff:eoff + 128],
                        v_sb[jb][:, 65 * h + 64:65 * h + 65],
                        start=False,
                        stop=(jb == ib),
                        skip_group_check=True,
                    )
                if jb // 4 == it and h == 3:
                    enq(1, lambda: tail_norm(it, jb, ctx_t, sums))

            def attention_quarter(it, fillers):
                f_i = [0]
                groups = (4 * it + 4) * H
                g_q = [0]

                def fill_pace():
                    # finish the fillers ~80% through the quarter
                    want = (g_q[0] * len(fillers) * 5) // (groups * 4)
                    while f_i[0] < min(want, len(fillers)):
                        fillers[f_i[0]]()
                        f_i[0] += 1

                # ctx banks: 2 i-blocks per [128, 512] bank; sums in their own
                # bank at col 4*ib4+h; group-openers emitted lazily on first
                # ctx write (see emit_ctx)
                st = {
                    "ctx": [
                        ctxp.tile([128, 512], F32, tag="ctx", name=f"ctx{it}_{i}")
                        for i in range(2)
                    ],
                    "sums": sump.tile([128, 16], F32, tag="sums", name=f"s{it}"),
                    "opened": False,
                }

                for jb in range(4 * it + 4):
                    diag = jb // 4 == it
                    cst = 512 * it + (min(128 * (jb % 4), 256) if diag else 0)
                    w = 512 * (it + 1) - cst
                    for h in range(H):
                        g_i[0] += 1
                        g_q[0] += 1
                        qh = qT[h // 2][64 * (h % 2):64 * (h % 2) + 64, :]
                        kh = kT[h // 2][64 * (h % 2):64 * (h % 2) + 64, :]
                        sc_t = scp.tile([128, 512], F32, tag="sc", name="sct")
                        nc.tensor.matmul(
                            sc_t[:, 0:w],
                            kh[:, jb * 128:(jb + 1) * 128],
                            qh[:, cst:cst + w],
                            start=True,
                            stop=True,
                        )
                        skip = 128 if (diag and jb % 4 == 3) else 0
                        e = wpool.tile([128, 512], BF16, tag="e", bufs=8, name="e")
                        emit_exp(e, sc_t, skip, w)
                        if diag:
                            moff = 128 if jb % 4 == 3 else 0
                            # first few masks run before the ut tile filler
                            # is emitted, so they must use affine_select
                            use_dve = mask_idx[0] >= 12 and mask_idx[0] % 2 == 0
                            if use_dve:
                                nc.vector.scalar_tensor_tensor(
                                    e[:, moff:moff + 128], e[:, moff:moff + 128],
                                    1.0, ut_b[:],
                                    op0=mybir.AluOpType.mult,
                                    op1=mybir.AluOpType.mult,
                                )
                            else:
                                nc.gpsimd.affine_select(
                                    e[:, moff:moff + 128], e[:, moff:moff + 128],
                                    pattern=[[1, 128]],
                                    compare_op=mybir.AluOpType.is_ge,
                                    fill=0.0,
                                    base=0,
                                    channel_multiplier=-1,
                                )
                            mask_idx[0] += 1
                        pend.append((jb, h, e, cst, it, st))
                        run_due()
                        if len(pend) > 2:
                            emit_ctx(pend.pop(0))
                        fill_pace()
                while f_i[0] < len(fillers):
                    fillers[f_i[0]]()
                    f_i[0] += 1

            # ---------- main schedule ----------
            # quarter 0 prerequisites up-front
            for ib in range(4):
                unit_xT(ib, alternate=True)
            for which in range(2):
                for blk in range(2):
                    unit_qk(0, which, blk)
            for jb in range(4):
                unit_v(jb)

            for it in range(IT):
                fillers = []
                if it == 0:
                    fillers.append(unit_ut)
                    fillers.append(unit_wo_conv)
                else:
                    # k projections + v for this quarter's new (diagonal)
                    # j-blocks are only needed in the back half of the quarter
                    for blk in range(2):
                        fillers.append(lambda b=blk, t=it: unit_qk(t, 1, b))
                    for jb in range(4 * it, 4 * it + 4):
                        fillers.append(lambda jb=jb: unit_v(jb))
                if it + 1 < IT:
                    for ib in range(4 * (it + 1), 4 * (it + 1) + 4):
                        fillers.append(lambda ib=ib: unit_xT(ib))
                    for blk in range(2):
                        fillers.append(
                            lambda b=blk, t=it + 1: unit_qk(t, 0, b)
                        )
                attention_quarter(it, fillers)

            # drain the pipeline and remaining tail stages
            while pend:
                g_i[0] += 1
                run_due()
                emit_ctx(pend.pop(0))
            while sched:
                g_i[0] += 1
                run_due()

    nc.compile()
    return nc


_STATE = {}


def _get_runner():
    if "run" in _STATE:
        return _STATE["run"]
    import jax
    from concourse.bass2jax import (
        _bass_exec_p,
        install_neuronx_cc_hook,
        partition_id_tensor,
    )
    from jax.sharding import Mesh, PartitionSpec
    from jax.experimental.shard_map import shard_map

    nc = build_program()
    install_neuronx_cc_hook()
    partition_name = nc.partition_id_tensor.name if nc.partition_id_tensor else None
    in_names, out_names, out_avals, zero_outs = [], [], [], []
    for alloc in nc.m.functions[0].allocations:
        if not isinstance(alloc, mybir.MemoryLocationSet):
            continue
        name = alloc.memorylocations[0].name
        if alloc.kind == "ExternalInput":
            if name != partition_name:
                in_names.append(name)
        elif alloc.kind == "ExternalOutput":
            shape = tuple(alloc.tensor_shape)
            dtype = mybir.dt.np(alloc.dtype)
            out_names.append(name)
            out_avals.append(jax.core.ShapedArray(shape, dtype))
            zero_outs.append(np.zeros(shape, dtype))
    n_params = len(in_names)
    all_in = in_names + out_names + ([partition_name] if partition_name else [])

    def _body(*args):
        operands = list(args)
        if partition_name is not None:
            operands.append(partition_id_tensor())
        return tuple(
            _bass_exec_p.bind(
                *operands,
                out_avals=tuple(out_avals),
                in_names=tuple(all_in),
                out_names=tuple(out_names),
                lowering_input_output_aliases=(),
                sim_require_finite=True,
                sim_require_nnan=True,
                nc=nc,
            )
        )

    devices = jax.devices()[:B]
    mesh = Mesh(np.asarray(devices), ("core",))
    specs = (PartitionSpec("core"),) * (n_params + len(out_names))
    jitted = jax.jit(
        shard_map(
            _body, mesh=mesh, in_specs=specs,
            out_specs=(PartitionSpec("core"),) * len(out_names), check_rep=False,
        ),
        keep_unused=True,
    )

    def run(in_maps):
        import jax as _jax

        concat_in = [
            np.concatenate([np.asarray(m[nm]) for m in in_maps], axis=0)
            for nm in in_names
        ]
        concat_zero = [
            np.zeros((B * z.shape[0], *z.shape[1:]), z.dtype) for z in zero_outs
        ]
        outs = jitted(*concat_in, *concat_zero)
        _jax.block_until_ready(outs)
        res = []
        o = np.asarray(outs[out_names.index("out")])
        per = o.shape[0] // B
        for c in range(B):
            res.append(o[c * per:(c + 1) * per])
        return res

    _STATE["run"] = run
    return run


def make_in_maps(node_features, Wq, bq, Wk, bk, Wv, bv, Wo, bo, Wu, bu):
    in_maps = []
    for c in range(B):
        in_maps.append(
            {
                "x": np.ascontiguousarray(node_features[c], dtype=np.float32),
                "wq": np.asarray(Wq, np.float32),
                "wk": np.asarray(Wk, np.float32),
                "wv": np.asarray(Wv, np.float32),
                "wo": np.asarray(Wo, np.float32),
                "wu": np.asarray(Wu, np.float32),
                "bq": np.asarray(bq, np.float32),
                "bk": np.asarray(bk, np.float32),
                "bv": np.asarray(bv, np.float32),
                "bo": np.asarray(bo, np.float32),
                "bu": np.asarray(bu, np.float32),
            }
        )
    return in_maps


def kernel(
    node_features, causal_mask, Wq, bq, Wk, bk, Wv, bv, Wo, bo, Wu, bu
):
    """Full-input entry point: shards batch across 8 cores internally."""
    del causal_mask  # guaranteed tril(ones); mask generated on-chip
    run = _get_runner()
    in_maps = make_in_maps(node_features, Wq, bq, Wk, bk, Wv, bv, Wo, bo, Wu, bu)
    outs = run(in_maps)
    return np.stack(outs, axis=0)


# revision 49
# speedup vs baseline: 1.0998x; 1.0998x over previous
"""Trainium2 Bass kernel for CausalMessagePassing (B=8, N=2048, D=256, H=4).

Strategy: data-parallel across 8 NeuronCores, one graph per core.
Per-core dataflow (v2 "row-major ctx" redesign):
  x -> x^T (PE transpose, bf16 identity); q^T,k^T col-major f32r; v
  row-major bf16 with a ones column per head.
  scores^T[j,i] = k_h^T.T @ q_h^T per head in f32r over the causal
  triangle only (mask input never DMA'd; tril structure exploited).
  e = exp(scores/8) -> bf16, split across engines: ACT (exact) plus
  Pool/DVE tiles using a Schraudolph bf16 approximation (int16 bitcast);
  softmax normalization cancels its systematic component.
  Diagonal-tile causal mask = multiply with a constant upper-tri bf16
  mask on DVE (4x mode).
  ctx row-major per i-block: ctx[i, h*65+c] accumulates e-chunk^T @
  [v_h | 1] with all 4 heads sharing one PSUM bank (K=1 zero-row opener
  matmul establishes the accumulation group + write ordering).
  Norm: DVE reciprocal of the 4 sums columns + broadcast multiply
  during PSUM eviction -> ctx_n bf16; PE-transpose (bf16) back to
  col-major ectx for the Wo spine.  messages^T = Wo.T @ ectx (+bo).
  u computed ROW-major: u[i,:] = relu(sum_c chunk_c(x^T,m^T).T @ Wu_c
  + bu) with bu folded in via a K=1 opener matmul; direct DMA out, no
  output transposes.
  Extras: PE HAM warm-up + ACT exp-table preload during the input DMA
  window; software-pipelined exp->ctx; projection/tail work interleaved
  into the attention stream to fill PE stall gaps.
"""
import sys

sys.path.insert(0, "/opt/trn_rl_repo")

import numpy as np

import concourse.bass as bass  # noqa: F401
import concourse.mybir as mybir
import concourse.tile as tile
from concourse import bacc
from concourse.masks import make_identity

B, N, DM, H = 8, 2048, 256, 4
HD = DM // H  # 64
NB = N // 128  # 16 i/j blocks
IT = N // 512  # 4 quarters
F32 = mybir.dt.float32
F32R = mybir.dt.float32r
BF16 = mybir.dt.bfloat16
I16 = mybir.dt.int16

# Schraudolph bf16 exp: i16 = trunc(x * SCHR_S + SCHR_B); bitcast -> bf16
SCHR_S = 128.0 / np.log(2.0)
SCHR_B = 127.0 * 128.0 - 5.1
SCALE = 0.125  # 1/sqrt(HD)

# exp-tile engine assignment pattern (A=ACT exact, P=Pool, D=DVE approx)
EXP_PATTERN = "AAAAAPPD"


def build_program():
    nc = bacc.Bacc("TRN2", target_bir_lowering=False, debug=False)
    x_d = nc.dram_tensor("x", [N, DM], F32, kind="ExternalInput").ap()
    wq_d = nc.dram_tensor("wq", [DM, DM], F32, kind="ExternalInput").ap()
    wk_d = nc.dram_tensor("wk", [DM, DM], F32, kind="ExternalInput").ap()
    wv_d = nc.dram_tensor("wv", [DM, DM], F32, kind="ExternalInput").ap()
    wo_d = nc.dram_tensor("wo", [DM, DM], F32, kind="ExternalInput").ap()
    wu_d = nc.dram_tensor("wu", [2 * DM, DM], F32, kind="ExternalInput").ap()
    bq_d = nc.dram_tensor("bq", [DM], F32, kind="ExternalInput").ap()
    bk_d = nc.dram_tensor("bk", [DM], F32, kind="ExternalInput").ap()
    bv_d = nc.dram_tensor("bv", [DM], F32, kind="ExternalInput").ap()
    bo_d = nc.dram_tensor("bo", [DM], F32, kind="ExternalInput").ap()
    bu_d = nc.dram_tensor("bu", [DM], F32, kind="ExternalInput").ap()
    out_d = nc.dram_tensor("out", [N, DM], F32, kind="ExternalOutput").ap()

    def r(ap):
        return ap.bitcast(F32R)

    with tile.TileContext(nc) as tc:
        with (
            tc.tile_pool(name="const", bufs=1) as cpool,
            tc.tile_pool(name="big", bufs=1) as bpool,
            tc.tile_pool(name="work", bufs=3) as wpool,
            tc.tile_pool(name="mm", bufs=1, space="PSUM") as mmp,
            tc.tile_pool(name="sc", bufs=4, space="PSUM") as scp,
            tc.tile_pool(name="ctxp", bufs=2, space="PSUM") as ctxp,
            tc.tile_pool(name="sump", bufs=1, space="PSUM") as sump,
        ):
            # ---- constants / weights (batched DMAs) ----
            ident_f = cpool.tile([128, 128], F32, tag="identf")
            make_identity(nc, ident_f[:])
            ident = cpool.tile([128, 128], F32R, tag="ident")
            nc.vector.tensor_copy(ident[:], ident_f[:])
            ident_b = cpool.tile([128, 128], BF16, tag="identb")
            nc.vector.tensor_copy(ident_b[:], ident_f[:])
            # PE HAM warm-up during the input-DMA window; ACT exp-table
            # preload off the critical path.
            warm = scp.tile([128, 512], F32R, tag="sc", name="warm")
            for _ in range(12):
                nc.tensor.transpose(warm[0:128, 0:128], ident[:], ident[:])
            wexp = cpool.tile([1, 8], F32, tag="wexp")
            nc.scalar.activation(
                wexp[:], ident_f[0:1, 0:8], mybir.ActivationFunctionType.Exp
            )
            # each W loaded as one DMA: [128, 2*DM], chunk c at cols [c*DM,..)
            wq_a = cpool.tile([128, 2 * DM], F32R, tag="wqa")
            wk_a = cpool.tile([128, 2 * DM], F32R, tag="wka")
            wv_a = cpool.tile([128, 2 * DM], F32R, tag="wva")
            wo_a = cpool.tile([128, 2 * DM], F32R, tag="woa")
            wu_a = cpool.tile([128, 4 * DM], F32R, tag="wua")

            def dma_w(t_sb, t_d):
                nc.sync.dma_start(
                    t_sb[:].rearrange("p (c d) -> p c d", d=DM),
                    r(t_d.rearrange("(c p) d -> p c d", p=128)),
                )

            stage = cpool.tile([128, NB * DM], F32R, tag="stage")
            xs_all = stage
            x_r = r(x_d.rearrange("(t p) d -> p t d", p=128))

            def dma_x(g):
                nc.sync.dma_start(
                    xs_all[:, g * 2 * DM:(g + 1) * 2 * DM].rearrange(
                        "p (t d) -> p t d", d=DM
                    ),
                    x_r[:, g * 2:(g + 1) * 2, :],
                )

            dma_x(0)
            dma_x(1)
            dma_w(wq_a, wq_d)
            dma_w(wk_a, wk_d)
            dma_w(wv_a, wv_d)
            wq_sb = [wq_a[:, c * DM:(c + 1) * DM] for c in range(2)]
            wk_sb = [wk_a[:, c * DM:(c + 1) * DM] for c in range(2)]
            wv_sb = [wv_a[:, c * DM:(c + 1) * DM] for c in range(2)]
            wu_sb = [wu_a[:, c * DM:(c + 1) * DM] for c in range(4)]
            bq_a = cpool.tile([128, 2], F32, tag="bqa")
            bk_a = cpool.tile([128, 2], F32, tag="bka")
            bo_a = cpool.tile([128, 2], F32, tag="boa")
            for t_sb, t_d in ((bq_a, bq_d), (bk_a, bk_d), (bo_a, bo_d)):
                nc.sync.dma_start(t_sb[:], t_d.rearrange("(c p) -> p c", p=128))
            bq_c = [bq_a[:, b:b + 1] for b in range(2)]
            bk_c = [bk_a[:, b:b + 1] for b in range(2)]
            bo_c = [bo_a[:, b:b + 1] for b in range(2)]
            # bu as a row [1, 256] for the K=1 bias-opener matmul
            bu_row = cpool.tile([1, DM], F32, tag="burow")
            nc.sync.dma_start(bu_row[:], bu_d.rearrange("(b a) -> b a", b=1))
            # bv broadcast tile [128, 256] (f32; used by DVE add)
            bv_row = cpool.tile([1, DM], F32, tag="bvrow")
            nc.sync.dma_start(bv_row[:], bv_d.rearrange("(b a) -> b a", b=1))
            ones1 = cpool.tile([1, 128], F32, tag="ones1")
            nc.gpsimd.memset(ones1[:], 1.0)
            bv_bc = cpool.tile([128, DM], F32, tag="bvbc")
            pt0 = mmp.tile([128, DM], F32, tag="mm")
            nc.tensor.matmul(pt0[:], ones1[:], bv_row[:], start=True, stop=True)
            nc.vector.tensor_copy(bv_bc[:], pt0[:])
            zrow = cpool.tile([1, 512], F32, tag="zrow")
            nc.gpsimd.memset(zrow[:], 0.0)
            ones_col4 = cpool.tile([128, 4], BF16, tag="onescol4")
            nc.gpsimd.memset(ones_col4[:], 1.0)
            # constant upper-tri bf16 mask (keep col >= row); built lazily so
            # it doesn't delay the startup Pool queue
            ut_b = cpool.tile([128, 128], BF16, tag="utb")

            def unit_ut():
                nc.gpsimd.memset(ut_b[:], 1.0)
                nc.gpsimd.affine_select(
                    ut_b[:], ut_b[:],
                    pattern=[[1, 128]],
                    compare_op=mybir.AluOpType.is_ge,
                    fill=0.0,
                    base=0,
                    channel_multiplier=-1,
                )
            # ---- remaining weights, then rest of x (x blocks 4+ are not
            # needed until the second quarter's transposes) ----
            dma_w(wo_a, wo_d)
            dma_w(wu_a, wu_d)
            for g in range(2, 8):
                dma_x(g)
            # Wo in bf16 so the m spine can use narrow (128-col) moving
            # tiles; conversion emitted lazily (as a quarter-0 filler) so it
            # doesn't head-block the DVE queue while the Wo DMA lands.
            wo_b = cpool.tile([128, 2 * DM], BF16, tag="wob")
            wo_sb = [wo_b[:, c * DM:(c + 1) * DM] for c in range(2)]

            def unit_wo_conv():
                nc.vector.tensor_copy(wo_b[:], wo_a[:])

            xT = [bpool.tile([128, N], F32R, tag=f"xT{c}", name=f"xT{c}") for c in range(2)]
            qT = [bpool.tile([128, N], F32R, tag=f"qT{b}", name=f"qT{b}") for b in range(2)]
            kT = [bpool.tile([128, N], F32R, tag=f"kT{b}", name=f"kT{b}") for b in range(2)]
            ectx = [bpool.tile([128, N], BF16, tag=f"ectx{b}", name=f"ectx{b}") for b in range(2)]
            m_sb = [bpool.tile([128, N], F32R, tag=f"m{b}", name=f"m{b}") for b in range(2)]
            # v_sb[jb]: [128, 4*65] bf16; head h data at 65h..65h+63, ones at 65h+64
            v_sb = [bpool.tile([128, 4 * 65], BF16, tag=f"v{jb}", name=f"v{jb}") for jb in range(NB)]
            ostage = stage
            out_r = r(out_d.rearrange("(t p) d -> p t d", p=128))

            # ---------- filler units (projection / tail work) ----------
            def unit_xT(ib, alternate=False):
                for c in range(2):
                    tp = scp.tile([128, 128], F32R, tag="sc", name="tp")
                    nc.tensor.transpose(
                        tp[:], xs_all[:, ib * DM + c * 128:ib * DM + (c + 1) * 128],
                        ident_b[:],
                    )
                    eng = (
                        nc.vector if alternate and (2 * ib + c) % 2 == 0
                        else nc.gpsimd
                    )
                    eng.tensor_copy(xT[c][:, ib * 128:(ib + 1) * 128], tp[:])

            def unit_qk(it, which, blk, pool=None):
                w_sb, b_c, dstT = (
                    (wq_sb, bq_c, qT) if which == 0 else (wk_sb, bk_c, kT)
                )
                pool = pool or mmp
                pt = pool.tile([128, 512], F32, tag=pool is mmp and "mm" or "sc", name="qkpt")
                for c in range(2):
                    nc.tensor.matmul(
                        pt[:],
                        w_sb[c][:, blk * 128:(blk + 1) * 128],
                        xT[c][:, it * 512:(it + 1) * 512],
                        start=(c == 0),
                        stop=(c == 1),
                    )
                nc.vector.tensor_scalar_add(
                    dstT[blk][:, it * 512:(it + 1) * 512], pt[:], b_c[blk][:]
                )

            def unit_v(jb, pool=None):
                v4 = v_sb[jb][:].rearrange("p (h e) -> p h e", e=65)
                nc.vector.tensor_copy(
                    v4[:, :, 64:65],
                    ones_col4[:].rearrange("p (h e) -> p h e", e=1),
                )
                pool = pool or mmp
                pt = pool.tile([128, DM], F32, tag=pool is mmp and "mm" or "sc", name="vpt")
                for c in range(2):
                    nc.tensor.matmul(
                        pt[:],
                        xT[c][:, jb * 128:(jb + 1) * 128],
                        wv_sb[c][:],
                        start=(c == 0),
                        stop=(c == 1),
                    )
                nc.vector.tensor_tensor(
                    v4[:, :, 0:64],
                    pt[:].rearrange("p (h e) -> p h e", e=64),
                    bv_bc[:].rearrange("p (h e) -> p h e", e=64),
                    op=mybir.AluOpType.add,
                )

            def unit_ctxT(ib, ctxn):
                for c in range(2):
                    tp = scp.tile([128, 128], BF16, tag="sc", name="tpc")
                    nc.tensor.transpose(
                        tp[:], ctxn[:, c * 128:(c + 1) * 128], ident_b[:]
                    )
                    nc.gpsimd.tensor_copy(ectx[c][:, ib * 128:(ib + 1) * 128], tp[:])

            def unit_m(ib, blk):
                # messages^T for one i-block (128 cols, bf16 moving)
                isl = slice(ib * 128, (ib + 1) * 128)
                pt = mmp.tile([128, 128], F32, tag="mm", name="mpt")
                for c in range(2):
                    nc.tensor.matmul(
                        pt[:],
                        wo_sb[c][:, blk * 128:(blk + 1) * 128],
                        ectx[c][:, isl],
                        start=(c == 0),
                        stop=(c == 1),
                    )
                nc.vector.tensor_scalar_add(m_sb[blk][:, isl], pt[:], bo_c[blk][:])

            def unit_u(ib):
                up = mmp.tile([128, DM], F32, tag="mm", name="up")
                # K=1 bias opener: writes bu to every row, claims the group
                nc.tensor.matmul(
                    up[:], r(ones1[:]), r(bu_row[:]), start=True, stop=False,
                    skip_group_check=True,
                )
                for c in range(4):
                    lhs = xT[c] if c < 2 else m_sb[c - 2]
                    nc.tensor.matmul(
                        up[:],
                        lhs[:, ib * 128:(ib + 1) * 128],
                        wu_sb[c],
                        start=False,
                        stop=(c == 3),
                        skip_group_check=True,
                    )
                nc.vector.tensor_scalar_max(
                    ostage[:, ib * DM:(ib + 1) * DM], up[:], 0.0
                )
                nc.sync.dma_start(
                    out_r[:, ib:ib + 1, :],
                    ostage[:, ib * DM:(ib + 1) * DM].rearrange(
                        "p (t d) -> p t d", d=DM
                    ),
                )

            # ---------- attention ----------
            exp_idx = [0]

            def emit_exp(e, sc_t, skip, w):
                kind = EXP_PATTERN[exp_idx[0] % len(EXP_PATTERN)]
                exp_idx[0] += 1
                if kind == "A":
                    nc.scalar.activation(
                        e[:, skip:w], sc_t[:, skip:w],
                        mybir.ActivationFunctionType.Exp,
                        scale=SCALE,
                    )
                else:
                    eng = nc.gpsimd if kind == "P" else nc.vector
                    eng.tensor_scalar(
                        e[:, skip:w].bitcast(I16), sc_t[:, skip:w],
                        float(SCHR_S * SCALE), float(SCHR_B),
                        op0=mybir.AluOpType.mult, op1=mybir.AluOpType.add,
                    )

            mask_idx = [0]

            # ---------- flat attention pipeline across all quarters ----------
            g_i = [0]      # global group counter
            sched = []     # (due_group, fn) deferred tail stages
            pend = []      # scores->ctx software pipeline entries

            def enq(delay, fn):
                sched.append((g_i[0] + delay, fn))

            def run_due():
                for item in list(sched):
                    if item[0] <= g_i[0]:
                        sched.remove(item)
                        item[1]()

            def tail_norm(it, ib, ctx_t, sums):
                ib4 = ib % 4
                rsb = wpool.tile([128, 4], F32, tag="rsb", bufs=2, name="rsb")
                nc.vector.reciprocal(
                    rsb[:].rearrange("p (h e) -> p h e", e=1),
                    sums[:].rearrange("p (h e) -> p h e", e=1)[:, 4 * ib4:4 * ib4 + 4],
                )
                c4 = ctx_t[ib4 // 2][:, 256 * (ib4 % 2):256 * (ib4 % 2) + 256]
                ctxn = wpool.tile([128, DM], BF16, tag="ctxn", bufs=4, name="ctxn")
                in0 = c4.rearrange("p (h e) -> p h e", e=64)
                in1 = rsb[:].rearrange("p (h e) -> p h e", e=1)
                in0b, in1b = bass.broadcast_tensor_aps(in0, in1)
                nc.vector.tensor_tensor(
                    ctxn[:].rearrange("p (h e) -> p h e", e=64),
                    in0b, in1b, op=mybir.AluOpType.mult,
                )
                enq(2, lambda: tail_ctxT(ib, ctxn))

            def tail_ctxT(ib, ctxn):
                unit_ctxT(ib, ctxn)
                enq(2, lambda: tail_m(ib))

            def tail_m(ib):
                for blk in range(2):
                    unit_m(ib, blk)
                enq(2, lambda: tail_u(ib))

            def tail_u(ib):
                unit_u(ib)

            def emit_ctx(ent):
                jb, h, e, cst, it, st = ent
                ctx_t, sums = st["ctx"], st["sums"]
                if not st["opened"]:
                    # deferred group-openers: must land after the previous
                    # quarter's norm reads of the same PSUM slots
                    st["opened"] = True
                    for i in range(2):
                        nc.tensor.matmul(
                            ctx_t[i][:], r(ones1[:]), r(zrow[:]), start=True,
                            stop=False, skip_group_check=True,
                        )
                    nc.tensor.matmul(
                        sums[:], r(ones1[:]), r(zrow[0:1, 0:16]), start=True,
                        stop=False, skip_group_check=True,
                    )
                for ib in range(max(4 * it, jb), 4 * it + 4):
                    ib4 = ib % 4
                    eoff = 128 * ib4 - (cst - 512 * it)
                    nc.tensor.matmul(
                        ctx_t[ib4 // 2][:, 256 * (ib4 % 2) + 64 * h:
                                        256 * (ib4 % 2) + 64 * h + 64],
                        e[:, eoff:eoff + 128],
                        v_sb[jb][:, 65 * h:65 * h + 64],
                        start=False,
                        stop=(jb == ib),
                        skip_group_check=True,
                    )
                    nc.tensor.matmul(
                        sums[:, 4 * ib4 + h:4 * ib4 + h + 1],
                        e[:, eoff:eoff + 128],
                        v_sb[jb][:, 65 * h + 64:65 * h + 65],
                        start=False,
                        stop=(jb == ib),
                        skip_group_check=True,
                    )
                if jb // 4 == it and h == 3:
                    enq(1, lambda: tail_norm(it, jb, ctx_t, sums))

            def attention_quarter(it, fillers):
                f_i = [0]
                groups = (4 * it + 4) * H
                g_q = [0]

                def fill_pace():
                    # finish the fillers ~80% through the quarter
                    want = (g_q[0] * len(fillers) * 5) // (groups * 4)
                    while f_i[0] < min(want, len(fillers)):
                        fillers[f_i[0]]()
                        f_i[0] += 1

                # ctx banks: 2 i-blocks per [128, 512] bank; sums in their own
                # bank at col 4*ib4+h; group-openers emitted lazily on first
                # ctx write (see emit_ctx)
                st = {
                    "ctx": [
                        ctxp.tile([128, 512], F32, tag="ctx", name=f"ctx{it}_{i}")
                        for i in range(2)
                    ],
                    "sums": sump.tile([128, 16], F32, tag="sums", name=f"s{it}"),
                    "opened": False,
                }

                for jb in range(4 * it + 4):
                    diag = jb // 4 == it
                    cst = 512 * it + (min(128 * (jb % 4), 256) if diag else 0)
                    w = 512 * (it + 1) - cst
                    for h in range(H):
                        g_i[0] += 1
                        g_q[0] += 1
                        qh = qT[h // 2][64 * (h % 2):64 * (h % 2) + 64, :]
                        kh = kT[h // 2][64 * (h % 2):64 * (h % 2) + 64, :]
                        sc_t = scp.tile([128, 512], F32, tag="sc", name="sct")
                        nc.tensor.matmul(
                            sc_t[:, 0:w],
                            kh[:, jb * 128:(jb + 1) * 128],
                            qh[:, cst:cst + w],
                            start=True,
                            stop=True,
                        )
                        skip = 128 if (diag and jb % 4 == 3) else 0
                        e = wpool.tile([128, 512], BF16, tag="e", bufs=8, name="e")
                        emit_exp(e, sc_t, skip, w)
                        if diag:
                            moff = 128 if jb % 4 == 3 else 0
                            # first few masks run before the ut tile filler
                            # is emitted, so they must use affine_select
                            use_dve = mask_idx[0] >= 12 and mask_idx[0] % 2 == 0
                            if use_dve:
                                nc.vector.scalar_tensor_tensor(
                                    e[:, moff:moff + 128], e[:, moff:moff + 128],
                                    1.0, ut_b[:],
                                    op0=mybir.AluOpType.mult,
                                    op1=mybir.AluOpType.mult,
                                )
                            else:
                                nc.gpsimd.affine_select(
                                    e[:, moff:moff + 128], e[:, moff:moff + 128],
                                    pattern=[[1, 128]],
                                    compare_op=mybir.AluOpType.is_ge,
                                    fill=0.0,
                                    base=0,
                                    channel_multiplier=-1,
                                )
                            mask_idx[0] += 1
                        pend.append((jb, h, e, cst, it, st))
                        run_due()
                        if len(pend) > 3:
                            emit_ctx(pend.pop(0))
                        fill_pace()
                while f_i[0] < len(fillers):
                    fillers[f_i[0]]()
                    f_i[0] += 1

            # ---------- main schedule ----------
            # quarter 0 prerequisites up-front
            for ib in range(4):
                unit_xT(ib, alternate=True)
            for which in range(2):
                for blk in range(2):
                    unit_qk(0, which, blk, pool=scp)
            for jb in range(4):
                unit_v(jb, pool=scp)

            for it in range(IT):
                fillers = []
                if it == 0:
                    fillers.append(unit_ut)
                    fillers.append(unit_wo_conv)
                else:
                    # k projections + v for this quarter's new (diagonal)
                    # j-blocks are only needed in the back half of the quarter
                    for blk in range(2):
                        fillers.append(lambda b=blk, t=it: unit_qk(t, 1, b))
                    for jb in range(4 * it, 4 * it + 4):
                        fillers.append(lambda jb=jb: unit_v(jb))
                if it + 1 < IT:
                    for ib in range(4 * (it + 1), 4 * (it + 1) + 4):
                        fillers.append(lambda ib=ib: unit_xT(ib))
                    for blk in range(2):
                        fillers.append(
                            lambda b=blk, t=it + 1: unit_qk(t, 0, b)
                        )
                attention_quarter(it, fillers)

            # drain the pipeline and remaining tail stages
            while pend:
                g_i[0] += 1
                run_due()
                emit_ctx(pend.pop(0))
            while sched:
                g_i[0] += 1
                run_due()

    nc.compile()
    return nc


_STATE = {}


def _get_runner():
    if "run" in _STATE:
        return _STATE["run"]
    import jax
    from concourse.bass2jax import (
        _bass_exec_p,
        install_neuronx_cc_hook,
        partition_id_tensor,
    )
    from jax.sharding import Mesh, PartitionSpec
    from jax.experimental.shard_map import shard_map

    nc = build_program()
    install_neuronx_cc_hook()
    partition_name = nc.partition_id_tensor.name if nc.partition_id_tensor else None
    in_names, out_names, out_avals, zero_outs = [], [], [], []
    for alloc in nc.m.functions[0].allocations:
        if not isinstance(alloc, mybir.MemoryLocationSet):
            continue
        name = alloc.memorylocations[0].name
        if alloc.kind == "ExternalInput":
            if name != partition_name:
                in_names.append(name)
        elif alloc.kind == "ExternalOutput":
            shape = tuple(alloc.tensor_shape)
            dtype = mybir.dt.np(alloc.dtype)
            out_names.append(name)
            out_avals.append(jax.core.ShapedArray(shape, dtype))
            zero_outs.append(np.zeros(shape, dtype))
    n_params = len(in_names)
    all_in = in_names + out_names + ([partition_name] if partition_name else [])

    def _body(*args):
        operands = list(args)
        if partition_name is not None:
            operands.append(partition_id_tensor())
        return tuple(
            _bass_exec_p.bind(
                *operands,
                out_avals=tuple(out_avals),
                in_names=tuple(all_in),
                out_names=tuple(out_names),
                lowering_input_output_aliases=(),
                sim_require_finite=True,
                sim_require_nnan=True,
                nc=nc,
            )
        )

    devices = jax.devices()[:B]
    mesh = Mesh(np.asarray(devices), ("core",))
    specs = (PartitionSpec("core"),) * (n_params + len(out_names))
    jitted = jax.jit(
        shard_map(
            _body, mesh=mesh, in_specs=specs,
            out_specs=(PartitionSpec("core"),) * len(out_names), check_rep=False,
        ),
        keep_unused=True,
    )

    def run(in_maps):
        import jax as _jax

        concat_in = [
            np.concatenate([np.asarray(m[nm]) for m in in_maps], axis=0)
            for nm in in_names
        ]
        concat_zero = [
            np.zeros((B * z.shape[0], *z.shape[1:]), z.dtype) for z in zero_outs
        ]
        outs = jitted(*concat_in, *concat_zero)
        _jax.block_until_ready(outs)
        res = []
        o = np.asarray(outs[out_names.index("out")])
        per = o.shape[0] // B
        for c in range(B):
            res.append(o[c * per:(c + 1) * per])
        return res

    _STATE["run"] = run
    return run


def make_in_maps(node_features, Wq, bq, Wk, bk, Wv, bv, Wo, bo, Wu, bu):
    in_maps = []
    for c in range(B):
        in_maps.append(
            {
                "x": np.ascontiguousarray(node_features[c], dtype=np.float32),
                "wq": np.asarray(Wq, np.float32),
                "wk": np.asarray(Wk, np.float32),
                "wv": np.asarray(Wv, np.float32),
                "wo": np.asarray(Wo, np.float32),
                "wu": np.asarray(Wu, np.float32),
                "bq": np.asarray(bq, np.float32),
                "bk": np.asarray(bk, np.float32),
                "bv": np.asarray(bv, np.float32),
                "bo": np.asarray(bo, np.float32),
                "bu": np.asarray(bu, np.float32),
            }
        )
    return in_maps


def kernel(
    node_features, causal_mask, Wq, bq, Wk, bk, Wv, bv, Wo, bo, Wu, bu
):
    """Full-input entry point: shards batch across 8 cores internally."""
    del causal_mask  # guaranteed tril(ones); mask generated on-chip
    run = _get_runner()
    in_maps = make_in_maps(node_features, Wq, bq, Wk, bk, Wv, bv, Wo, bo, Wu, bu)
    outs = run(in_maps)
    return np.stack(outs, axis=0)


# revision 51
# speedup vs baseline: 1.1835x; 1.0761x over previous
"""Trainium2 Bass kernel for CausalMessagePassing (B=8, N=2048, D=256, H=4).

Strategy: data-parallel across 8 NeuronCores, one graph per core.
Per-core dataflow (v2 "row-major ctx" redesign):
  x -> x^T (PE transpose, bf16 identity); q^T,k^T col-major f32r; v
  row-major bf16 with a ones column per head.
  scores^T[j,i] = k_h^T.T @ q_h^T per head in f32r over the causal
  triangle only (mask input never DMA'd; tril structure exploited).
  e = exp(scores/8) -> bf16, split across engines: ACT (exact) plus
  Pool/DVE tiles using a Schraudolph bf16 approximation (int16 bitcast);
  softmax normalization cancels its systematic component.
  Diagonal-tile causal mask = multiply with a constant upper-tri bf16
  mask on DVE (4x mode).
  ctx row-major per i-block: ctx[i, h*65+c] accumulates e-chunk^T @
  [v_h | 1] with all 4 heads sharing one PSUM bank (K=1 zero-row opener
  matmul establishes the accumulation group + write ordering).
  Norm: DVE reciprocal of the 4 sums columns + broadcast multiply
  during PSUM eviction -> ctx_n bf16; PE-transpose (bf16) back to
  col-major ectx for the Wo spine.  messages^T = Wo.T @ ectx (+bo).
  u computed ROW-major: u[i,:] = relu(sum_c chunk_c(x^T,m^T).T @ Wu_c
  + bu) with bu folded in via a K=1 opener matmul; direct DMA out, no
  output transposes.
  Extras: PE HAM warm-up + ACT exp-table preload during the input DMA
  window; software-pipelined exp->ctx; projection/tail work interleaved
  into the attention stream to fill PE stall gaps.
"""
import sys

sys.path.insert(0, "/opt/trn_rl_repo")

import numpy as np

import concourse.bass as bass  # noqa: F401
import concourse.mybir as mybir
import concourse.tile as tile
from concourse import bacc
from concourse.masks import make_identity

B, N, DM, H = 8, 2048, 256, 4
HD = DM // H  # 64
NB = N // 128  # 16 i/j blocks
IT = N // 512  # 4 quarters
F32 = mybir.dt.float32
F32R = mybir.dt.float32r
BF16 = mybir.dt.bfloat16
I16 = mybir.dt.int16

# Schraudolph bf16 exp: i16 = trunc(x * SCHR_S + SCHR_B); bitcast -> bf16
SCHR_S = 128.0 / np.log(2.0)
SCHR_B = 127.0 * 128.0 - 5.1
SCALE = 0.125  # 1/sqrt(HD)

import os

# exp-tile engine assignment pattern (A=ACT exact, P=Pool, D=DVE approx)
EXP_PATTERN = os.environ.get("K_EXP_PATTERN", "AAAAAPPD")
PEND_DEPTH = int(os.environ.get("K_PEND_DEPTH", "3"))
WARM_N = int(os.environ.get("K_WARM_N", "12"))
TAIL_D1 = int(os.environ.get("K_TAIL_D1", "1"))
TAIL_D2 = int(os.environ.get("K_TAIL_D2", "2"))
FILL_NUM = int(os.environ.get("K_FILL_NUM", "5"))
FILL_DEN = int(os.environ.get("K_FILL_DEN", "4"))
MASK_DVE_MOD = int(os.environ.get("K_MASK_DVE_MOD", "2"))
import json as _json
EXP_BIAS = _json.loads(os.environ.get("K_EXP_BIAS", '{"A":1.0,"D":1.0,"P":1.0}'))


def build_program():
    nc = bacc.Bacc("TRN2", target_bir_lowering=False, debug=False)
    x_d = nc.dram_tensor("x", [N, DM], F32, kind="ExternalInput").ap()
    wq_d = nc.dram_tensor("wq", [DM, DM], F32, kind="ExternalInput").ap()
    wk_d = nc.dram_tensor("wk", [DM, DM], F32, kind="ExternalInput").ap()
    wv_d = nc.dram_tensor("wv", [DM, DM], F32, kind="ExternalInput").ap()
    wo_d = nc.dram_tensor("wo", [DM, DM], F32, kind="ExternalInput").ap()
    wu_d = nc.dram_tensor("wu", [2 * DM, DM], F32, kind="ExternalInput").ap()
    bq_d = nc.dram_tensor("bq", [DM], F32, kind="ExternalInput").ap()
    bk_d = nc.dram_tensor("bk", [DM], F32, kind="ExternalInput").ap()
    bv_d = nc.dram_tensor("bv", [DM], F32, kind="ExternalInput").ap()
    bo_d = nc.dram_tensor("bo", [DM], F32, kind="ExternalInput").ap()
    bu_d = nc.dram_tensor("bu", [DM], F32, kind="ExternalInput").ap()
    out_d = nc.dram_tensor("out", [N, DM], F32, kind="ExternalOutput").ap()

    def r(ap):
        return ap.bitcast(F32R)

    with tile.TileContext(nc) as tc:
        with (
            tc.tile_pool(name="const", bufs=1) as cpool,
            tc.tile_pool(name="big", bufs=1) as bpool,
            tc.tile_pool(name="work", bufs=3) as wpool,
            tc.tile_pool(name="mm", bufs=1, space="PSUM") as mmp,
            tc.tile_pool(name="sc", bufs=4, space="PSUM") as scp,
            tc.tile_pool(name="ctxp", bufs=2, space="PSUM") as ctxp,
            tc.tile_pool(name="sump", bufs=1, space="PSUM") as sump,
        ):
            # ---- constants / weights (batched DMAs) ----
            ident_f = cpool.tile([128, 128], F32, tag="identf")
            make_identity(nc, ident_f[:])
            ident = cpool.tile([128, 128], F32R, tag="ident")
            nc.vector.tensor_copy(ident[:], ident_f[:])
            ident_b = cpool.tile([128, 128], BF16, tag="identb")
            nc.vector.tensor_copy(ident_b[:], ident_f[:])
            # PE HAM warm-up during the input-DMA window; ACT exp-table
            # preload off the critical path.
            warm = scp.tile([128, 512], F32R, tag="sc", name="warm")
            for _ in range(WARM_N):
                nc.tensor.transpose(warm[0:128, 0:128], ident[:], ident[:])
            wexp = cpool.tile([1, 8], F32, tag="wexp")
            nc.scalar.activation(
                wexp[:], ident_f[0:1, 0:8], mybir.ActivationFunctionType.Exp
            )
            # each W loaded as one DMA: [128, 2*DM], chunk c at cols [c*DM,..)
            wq_a = cpool.tile([128, 2 * DM], F32R, tag="wqa")
            wk_a = cpool.tile([128, 2 * DM], F32R, tag="wka")
            wv_a = cpool.tile([128, 2 * DM], F32R, tag="wva")
            wo_a = cpool.tile([128, 2 * DM], F32R, tag="woa")
            wu_a = cpool.tile([128, 4 * DM], F32R, tag="wua")

            def dma_w(t_sb, t_d):
                nc.sync.dma_start(
                    t_sb[:].rearrange("p (c d) -> p c d", d=DM),
                    r(t_d.rearrange("(c p) d -> p c d", p=128)),
                )

            stage = cpool.tile([128, NB * DM], F32R, tag="stage")
            xs_all = stage
            x_r = r(x_d.rearrange("(t p) d -> p t d", p=128))

            def dma_x(g):
                nc.sync.dma_start(
                    xs_all[:, g * 2 * DM:(g + 1) * 2 * DM].rearrange(
                        "p (t d) -> p t d", d=DM
                    ),
                    x_r[:, g * 2:(g + 1) * 2, :],
                )

            dma_x(0)
            dma_x(1)
            dma_w(wq_a, wq_d)
            dma_w(wk_a, wk_d)
            dma_w(wv_a, wv_d)
            wq_sb = [wq_a[:, c * DM:(c + 1) * DM] for c in range(2)]
            wk_sb = [wk_a[:, c * DM:(c + 1) * DM] for c in range(2)]
            wv_sb = [wv_a[:, c * DM:(c + 1) * DM] for c in range(2)]
            wu_sb = [wu_a[:, c * DM:(c + 1) * DM] for c in range(4)]
            bq_a = cpool.tile([128, 2], F32, tag="bqa")
            bk_a = cpool.tile([128, 2], F32, tag="bka")
            bo_a = cpool.tile([128, 2], F32, tag="boa")
            for t_sb, t_d in ((bq_a, bq_d), (bk_a, bk_d), (bo_a, bo_d)):
                nc.sync.dma_start(t_sb[:], t_d.rearrange("(c p) -> p c", p=128))
            bq_c = [bq_a[:, b:b + 1] for b in range(2)]
            bk_c = [bk_a[:, b:b + 1] for b in range(2)]
            bo_c = [bo_a[:, b:b + 1] for b in range(2)]
            # bu as a row [1, 256] for the K=1 bias-opener matmul
            bu_row = cpool.tile([1, DM], F32, tag="burow")
            nc.sync.dma_start(bu_row[:], bu_d.rearrange("(b a) -> b a", b=1))
            # bv broadcast tile [128, 256] (f32; used by DVE add)
            bv_row = cpool.tile([1, DM], F32, tag="bvrow")
            nc.sync.dma_start(bv_row[:], bv_d.rearrange("(b a) -> b a", b=1))
            ones1 = cpool.tile([1, 128], F32, tag="ones1")
            nc.gpsimd.memset(ones1[:], 1.0)
            bv_bc = cpool.tile([128, DM], F32, tag="bvbc")
            pt0 = mmp.tile([128, DM], F32, tag="mm")
            nc.tensor.matmul(pt0[:], ones1[:], bv_row[:], start=True, stop=True)
            nc.vector.tensor_copy(bv_bc[:], pt0[:])
            zrow = cpool.tile([1, 512], F32, tag="zrow")
            nc.gpsimd.memset(zrow[:], 0.0)
            ones_col4 = cpool.tile([128, 4], BF16, tag="onescol4")
            nc.gpsimd.memset(ones_col4[:], 1.0)
            # constant upper-tri bf16 mask (keep col >= row); built lazily so
            # it doesn't delay the startup Pool queue
            ut_b = cpool.tile([128, 128], BF16, tag="utb")

            def unit_ut():
                nc.gpsimd.memset(ut_b[:], 1.0)
                nc.gpsimd.affine_select(
                    ut_b[:], ut_b[:],
                    pattern=[[1, 128]],
                    compare_op=mybir.AluOpType.is_ge,
                    fill=0.0,
                    base=0,
                    channel_multiplier=-1,
                )
            # ---- remaining weights, then rest of x (x blocks 4+ are not
            # needed until the second quarter's transposes) ----
            dma_w(wo_a, wo_d)
            dma_w(wu_a, wu_d)
            for g in range(2, 8):
                dma_x(g)
            # Wo in bf16 so the m spine can use narrow (128-col) moving
            # tiles; conversion emitted lazily (as a quarter-0 filler) so it
            # doesn't head-block the DVE queue while the Wo DMA lands.
            wo_b = cpool.tile([128, 2 * DM], BF16, tag="wob")
            wo_sb = [wo_b[:, c * DM:(c + 1) * DM] for c in range(2)]

            def unit_wo_conv():
                nc.vector.tensor_copy(wo_b[:], wo_a[:])

            xT = [bpool.tile([128, N], F32R, tag=f"xT{c}", name=f"xT{c}") for c in range(2)]
            qT = [bpool.tile([128, N], F32R, tag=f"qT{b}", name=f"qT{b}") for b in range(2)]
            kT = [bpool.tile([128, N], F32R, tag=f"kT{b}", name=f"kT{b}") for b in range(2)]
            ectx = [bpool.tile([128, N], BF16, tag=f"ectx{b}", name=f"ectx{b}") for b in range(2)]
            m_sb = [bpool.tile([128, N], F32R, tag=f"m{b}", name=f"m{b}") for b in range(2)]
            # v_sb[jb]: [128, 4*65] bf16; head h data at 65h..65h+63, ones at 65h+64
            v_sb = [bpool.tile([128, 4 * 65], BF16, tag=f"v{jb}", name=f"v{jb}") for jb in range(NB)]
            ostage = stage
            out_r = r(out_d.rearrange("(t p) d -> p t d", p=128))

            # ---------- filler units (projection / tail work) ----------
            def unit_xT(ib, alternate=False):
                for c in range(2):
                    tp = scp.tile([128, 128], F32R, tag="sc", name="tp")
                    nc.tensor.transpose(
                        tp[:], xs_all[:, ib * DM + c * 128:ib * DM + (c + 1) * 128],
                        ident_b[:],
                    )
                    if load["D"] + 258 < load["P"] + 273:
                        eng, k, cost = nc.vector, "D", 258
                    else:
                        eng, k, cost = nc.gpsimd, "P", 273
                    load[k] += cost
                    eng.tensor_copy(xT[c][:, ib * 128:(ib + 1) * 128], tp[:])

            def unit_qk(it, which, blk, pool=None):
                w_sb, b_c, dstT = (
                    (wq_sb, bq_c, qT) if which == 0 else (wk_sb, bk_c, kT)
                )
                pool = pool or mmp
                pt = pool.tile([128, 512], F32, tag=pool is mmp and "mm" or "sc", name="qkpt")
                for c in range(2):
                    nc.tensor.matmul(
                        pt[:],
                        w_sb[c][:, blk * 128:(blk + 1) * 128],
                        xT[c][:, it * 512:(it + 1) * 512],
                        start=(c == 0),
                        stop=(c == 1),
                    )
                load["D"] += 658
                nc.vector.tensor_scalar_add(
                    dstT[blk][:, it * 512:(it + 1) * 512], pt[:], b_c[blk][:]
                )

            def unit_v(jb, pool=None):
                v4 = v_sb[jb][:].rearrange("p (h e) -> p h e", e=65)
                nc.vector.tensor_copy(
                    v4[:, :, 64:65],
                    ones_col4[:].rearrange("p (h e) -> p h e", e=1),
                )
                pool = pool or mmp
                pt = pool.tile([128, DM], F32, tag=pool is mmp and "mm" or "sc", name="vpt")
                for c in range(2):
                    nc.tensor.matmul(
                        pt[:],
                        xT[c][:, jb * 128:(jb + 1) * 128],
                        wv_sb[c][:],
                        start=(c == 0),
                        stop=(c == 1),
                    )
                load["D"] += 420
                nc.vector.tensor_tensor(
                    v4[:, :, 0:64],
                    pt[:].rearrange("p (h e) -> p h e", e=64),
                    bv_bc[:].rearrange("p (h e) -> p h e", e=64),
                    op=mybir.AluOpType.add,
                )

            def unit_ctxT(ib, ctxn):
                for c in range(2):
                    tp = scp.tile([128, 128], BF16, tag="sc", name="tpc")
                    nc.tensor.transpose(
                        tp[:], ctxn[:, c * 128:(c + 1) * 128], ident_b[:]
                    )
                    if load["D"] + 258 < load["P"] + 273:
                        eng, k, cost = nc.vector, "D", 258
                    else:
                        eng, k, cost = nc.gpsimd, "P", 273
                    load[k] += cost
                    eng.tensor_copy(ectx[c][:, ib * 128:(ib + 1) * 128], tp[:])

            def unit_m(ib, blk):
                # messages^T for one i-block (128 cols, bf16 moving)
                isl = slice(ib * 128, (ib + 1) * 128)
                pt = mmp.tile([128, 128], F32, tag="mm", name="mpt")
                for c in range(2):
                    nc.tensor.matmul(
                        pt[:],
                        wo_sb[c][:, blk * 128:(blk + 1) * 128],
                        ectx[c][:, isl],
                        start=(c == 0),
                        stop=(c == 1),
                    )
                load["D"] += 258
                nc.vector.tensor_scalar_add(m_sb[blk][:, isl], pt[:], bo_c[blk][:])

            def unit_u(ib):
                up = mmp.tile([128, DM], F32, tag="mm", name="up")
                # K=1 bias opener: writes bu to every row, claims the group
                nc.tensor.matmul(
                    up[:], r(ones1[:]), r(bu_row[:]), start=True, stop=False,
                    skip_group_check=True,
                )
                for c in range(4):
                    lhs = xT[c] if c < 2 else m_sb[c - 2]
                    nc.tensor.matmul(
                        up[:],
                        lhs[:, ib * 128:(ib + 1) * 128],
                        wu_sb[c],
                        start=False,
                        stop=(c == 3),
                        skip_group_check=True,
                    )
                load["D"] += 391
                nc.vector.tensor_scalar_max(
                    ostage[:, ib * DM:(ib + 1) * DM], up[:], 0.0
                )
                nc.sync.dma_start(
                    out_r[:, ib:ib + 1, :],
                    ostage[:, ib * DM:(ib + 1) * DM].rearrange(
                        "p (t d) -> p t d", d=DM
                    ),
                )

            # ---------- attention ----------
            # greedy static load balancing: route each flexible op to the
            # engine with the least accumulated estimated busy time
            load = {"A": 0.0, "D": 0.0, "P": 0.0}

            def emit_exp(e, sc_t, skip, w):
                n = w - skip
                cost = {
                    "A": 0.833 * n + 215,
                    "D": 1.04 * n + 155,
                    "P": 1.39 * n + 125,
                }
                kind = min(("A", "D", "P"), key=lambda k: load[k] + cost[k] * EXP_BIAS[k])
                load[kind] += cost[kind]
                if kind == "A":
                    nc.scalar.activation(
                        e[:, skip:w], sc_t[:, skip:w],
                        mybir.ActivationFunctionType.Exp,
                        scale=SCALE,
                    )
                else:
                    eng = nc.gpsimd if kind == "P" else nc.vector
                    eng.tensor_scalar(
                        e[:, skip:w].bitcast(I16), sc_t[:, skip:w],
                        float(SCHR_S * SCALE), float(SCHR_B),
                        op0=mybir.AluOpType.mult, op1=mybir.AluOpType.add,
                    )

            mask_idx = [0]

            # ---------- flat attention pipeline across all quarters ----------
            g_i = [0]      # global group counter
            sched = []     # (due_group, fn) deferred tail stages
            pend = []      # scores->ctx software pipeline entries

            def enq(delay, fn):
                sched.append((g_i[0] + delay, fn))

            def run_due():
                for item in list(sched):
                    if item[0] <= g_i[0]:
                        sched.remove(item)
                        item[1]()

            def tail_norm(it, ib, ctx_t, sums):
                ib4 = ib % 4
                rsb = wpool.tile([128, 4], F32, tag="rsb", bufs=2, name="rsb")
                nc.vector.reciprocal(
                    rsb[:].rearrange("p (h e) -> p h e", e=1),
                    sums[:].rearrange("p (h e) -> p h e", e=1)[:, 4 * ib4:4 * ib4 + 4],
                )
                c4 = ctx_t[ib4 // 2][:, 256 * (ib4 % 2):256 * (ib4 % 2) + 256]
                ctxn = wpool.tile([128, DM], BF16, tag="ctxn", bufs=4, name="ctxn")
                in0 = c4.rearrange("p (h e) -> p h e", e=64)
                in1 = rsb[:].rearrange("p (h e) -> p h e", e=1)
                in0b, in1b = bass.broadcast_tensor_aps(in0, in1)
                load["D"] += 530
                nc.vector.tensor_tensor(
                    ctxn[:].rearrange("p (h e) -> p h e", e=64),
                    in0b, in1b, op=mybir.AluOpType.mult,
                )
                enq(TAIL_D2, lambda: tail_ctxT(ib, ctxn))

            def tail_ctxT(ib, ctxn):
                unit_ctxT(ib, ctxn)
                enq(TAIL_D2, lambda: tail_m(ib))

            def tail_m(ib):
                for blk in range(2):
                    unit_m(ib, blk)
                enq(TAIL_D2, lambda: tail_u(ib))

            def tail_u(ib):
                unit_u(ib)

            def emit_ctx(ent):
                jb, h, e, cst, it, st = ent
                ctx_t, sums = st["ctx"], st["sums"]
                if not st["opened"]:
                    # deferred group-openers: must land after the previous
                    # quarter's norm reads of the same PSUM slots
                    st["opened"] = True
                    for i in range(2):
                        nc.tensor.matmul(
                            ctx_t[i][:], r(ones1[:]), r(zrow[:]), start=True,
                            stop=False, skip_group_check=True,
                        )
                    nc.tensor.matmul(
                        sums[:], r(ones1[:]), r(zrow[0:1, 0:16]), start=True,
                        stop=False, skip_group_check=True,
                    )
                for ib in range(max(4 * it, jb), 4 * it + 4):
                    ib4 = ib % 4
                    eoff = 128 * ib4 - (cst - 512 * it)
                    nc.tensor.matmul(
                        ctx_t[ib4 // 2][:, 256 * (ib4 % 2) + 64 * h:
                                        256 * (ib4 % 2) + 64 * h + 64],
                        e[:, eoff:eoff + 128],
                        v_sb[jb][:, 65 * h:65 * h + 64],
                        start=False,
                        stop=(jb == ib),
                        skip_group_check=True,
                    )
                    nc.tensor.matmul(
                        sums[:, 4 * ib4 + h:4 * ib4 + h + 1],
                        e[:, eoff:eoff + 128],
                        v_sb[jb][:, 65 * h + 64:65 * h + 65],
                        start=False,
                        stop=(jb == ib),
                        skip_group_check=True,
                    )
                if jb // 4 == it and h == 3:
                    enq(TAIL_D1, lambda: tail_norm(it, jb, ctx_t, sums))

            def attention_quarter(it, fillers):
                f_i = [0]
                groups = (4 * it + 4) * H
                g_q = [0]

                def fill_pace():
                    # finish the fillers ~80% through the quarter
                    want = (g_q[0] * len(fillers) * FILL_NUM) // (groups * FILL_DEN)
                    while f_i[0] < min(want, len(fillers)):
                        fillers[f_i[0]]()
                        f_i[0] += 1

                # ctx banks: 2 i-blocks per [128, 512] bank; sums in their own
                # bank at col 4*ib4+h; group-openers emitted lazily on first
                # ctx write (see emit_ctx)
                st = {
                    "ctx": [
                        ctxp.tile([128, 512], F32, tag="ctx", name=f"ctx{it}_{i}")
                        for i in range(2)
                    ],
                    "sums": sump.tile([128, 16], F32, tag="sums", name=f"s{it}"),
                    "opened": False,
                }

                for jb in range(4 * it + 4):
                    diag = jb // 4 == it
                    cst = 512 * it + (min(128 * (jb % 4), 256) if diag else 0)
                    w = 512 * (it + 1) - cst
                    for h in range(H):
                        g_i[0] += 1
                        g_q[0] += 1
                        qh = qT[h // 2][64 * (h % 2):64 * (h % 2) + 64, :]
                        kh = kT[h // 2][64 * (h % 2):64 * (h % 2) + 64, :]
                        sc_t = scp.tile([128, 512], F32, tag="sc", name="sct")
                        nc.tensor.matmul(
                            sc_t[:, 0:w],
                            kh[:, jb * 128:(jb + 1) * 128],
                            qh[:, cst:cst + w],
                            start=True,
                            stop=True,
                        )
                        skip = 128 if (diag and jb % 4 == 3) else 0
                        e = wpool.tile([128, 512], BF16, tag="e", bufs=8, name="e")
                        emit_exp(e, sc_t, skip, w)
                        if diag:
                            moff = 128 if jb % 4 == 3 else 0
                            # first few masks run before the ut tile filler
                            # is emitted, so they must use affine_select
                            use_dve = mask_idx[0] >= 12 and (
                                load["D"] + 190 < load["P"] + 317
                            )
                            load["D" if use_dve else "P"] += 190 if use_dve else 317
                            if use_dve:
                                nc.vector.scalar_tensor_tensor(
                                    e[:, moff:moff + 128], e[:, moff:moff + 128],
                                    1.0, ut_b[:],
                                    op0=mybir.AluOpType.mult,
                                    op1=mybir.AluOpType.mult,
                                )
                            else:
                                nc.gpsimd.affine_select(
                                    e[:, moff:moff + 128], e[:, moff:moff + 128],
                                    pattern=[[1, 128]],
                                    compare_op=mybir.AluOpType.is_ge,
                                    fill=0.0,
                                    base=0,
                                    channel_multiplier=-1,
                                )
                            mask_idx[0] += 1
                        pend.append((jb, h, e, cst, it, st))
                        run_due()
                        if len(pend) > PEND_DEPTH:
                            emit_ctx(pend.pop(0))
                        fill_pace()
                while f_i[0] < len(fillers):
                    fillers[f_i[0]]()
                    f_i[0] += 1

            # ---------- main schedule ----------
            # quarter 0 prerequisites up-front
            for ib in range(4):
                unit_xT(ib, alternate=True)
            for which in range(2):
                for blk in range(2):
                    unit_qk(0, which, blk, pool=scp)
            for jb in range(4):
                unit_v(jb, pool=scp)

            for it in range(IT):
                fillers = []
                if it == 0:
                    fillers.append(unit_ut)
                    fillers.append(unit_wo_conv)
                else:
                    # k projections + v for this quarter's new (diagonal)
                    # j-blocks are only needed in the back half of the quarter
                    for blk in range(2):
                        fillers.append(lambda b=blk, t=it: unit_qk(t, 1, b))
                    for jb in range(4 * it, 4 * it + 4):
                        fillers.append(lambda jb=jb: unit_v(jb))
                if it + 1 < IT:
                    for ib in range(4 * (it + 1), 4 * (it + 1) + 4):
                        fillers.append(lambda ib=ib: unit_xT(ib))
                    for blk in range(2):
                        fillers.append(
                            lambda b=blk, t=it + 1: unit_qk(t, 0, b)
                        )
                attention_quarter(it, fillers)

            # drain the pipeline and remaining tail stages
            while pend:
                g_i[0] += 1
                run_due()
                emit_ctx(pend.pop(0))
            while sched:
                g_i[0] += 1
                run_due()

    nc.compile()
    return nc


_STATE = {}


def _get_runner():
    if "run" in _STATE:
        return _STATE["run"]
    import jax
    from concourse.bass2jax import (
        _bass_exec_p,
        install_neuronx_cc_hook,
        partition_id_tensor,
    )
    from jax.sharding import Mesh, PartitionSpec
    from jax.experimental.shard_map import shard_map

    nc = build_program()
    install_neuronx_cc_hook()
    partition_name = nc.partition_id_tensor.name if nc.partition_id_tensor else None
    in_names, out_names, out_avals, zero_outs = [], [], [], []
    for alloc in nc.m.functions[0].allocations:
        if not isinstance(alloc, mybir.MemoryLocationSet):
            continue
        name = alloc.memorylocations[0].name
        if alloc.kind == "ExternalInput":
            if name != partition_name:
                in_names.append(name)
        elif alloc.kind == "ExternalOutput":
            shape = tuple(alloc.tensor_shape)
            dtype = mybir.dt.np(alloc.dtype)
            out_names.append(name)
            out_avals.append(jax.core.ShapedArray(shape, dtype))
            zero_outs.append(np.zeros(shape, dtype))
    n_params = len(in_names)
    all_in = in_names + out_names + ([partition_name] if partition_name else [])

    def _body(*args):
        operands = list(args)
        if partition_name is not None:
            operands.append(partition_id_tensor())
        return tuple(
            _bass_exec_p.bind(
                *operands,
                out_avals=tuple(out_avals),
                in_names=tuple(all_in),
                out_names=tuple(out_names),
                lowering_input_output_aliases=(),
                sim_require_finite=True,
                sim_require_nnan=True,
                nc=nc,
            )
        )

    devices = jax.devices()[:B]
    mesh = Mesh(np.asarray(devices), ("core",))
    specs = (PartitionSpec("core"),) * (n_params + len(out_names))
    jitted = jax.jit(
        shard_map(
            _body, mesh=mesh, in_specs=specs,
            out_specs=(PartitionSpec("core"),) * len(out_names), check_rep=False,
        ),
        keep_unused=True,
    )

    def run(in_maps):
        import jax as _jax

        concat_in = [
            np.concatenate([np.asarray(m[nm]) for m in in_maps], axis=0)
            for nm in in_names
        ]
        concat_zero = [
            np.zeros((B * z.shape[0], *z.shape[1:]), z.dtype) for z in zero_outs
        ]
        outs = jitted(*concat_in, *concat_zero)
        _jax.block_until_ready(outs)
        res = []
        o = np.asarray(outs[out_names.index("out")])
        per = o.shape[0] // B
        for c in range(B):
            res.append(o[c * per:(c + 1) * per])
        return res

    _STATE["run"] = run
    return run


def make_in_maps(node_features, Wq, bq, Wk, bk, Wv, bv, Wo, bo, Wu, bu):
    in_maps = []
    for c in range(B):
        in_maps.append(
            {
                "x": np.ascontiguousarray(node_features[c], dtype=np.float32),
                "wq": np.asarray(Wq, np.float32),
                "wk": np.asarray(Wk, np.float32),
                "wv": np.asarray(Wv, np.float32),
                "wo": np.asarray(Wo, np.float32),
                "wu": np.asarray(Wu, np.float32),
                "bq": np.asarray(bq, np.float32),
                "bk": np.asarray(bk, np.float32),
                "bv": np.asarray(bv, np.float32),
                "bo": np.asarray(bo, np.float32),
                "bu": np.asarray(bu, np.float32),
            }
        )
    return in_maps


def kernel(
    node_features, causal_mask, Wq, bq, Wk, bk, Wv, bv, Wo, bo, Wu, bu
):
    """Full-input entry point: shards batch across 8 cores internally."""
    del causal_mask  # guaranteed tril(ones); mask generated on-chip
    run = _get_runner()
    in_maps = make_in_maps(node_features, Wq, bq, Wk, bk, Wv, bv, Wo, bo, Wu, bu)
    outs = run(in_maps)
    return np.stack(outs, axis=0)


# revision 54
# speedup vs baseline: 1.2014x; 1.0151x over previous
"""Trainium2 Bass kernel for CausalMessagePassing (B=8, N=2048, D=256, H=4).

Strategy: data-parallel across 8 NeuronCores, one graph per core.
Per-core dataflow (v2 "row-major ctx" redesign):
  x -> x^T (PE transpose, bf16 identity); q^T,k^T col-major f32r; v
  row-major bf16 with a ones column per head.
  scores^T[j,i] = k_h^T.T @ q_h^T per head in f32r over the causal
  triangle only (mask input never DMA'd; tril structure exploited).
  e = exp(scores/8) -> bf16, split across engines: ACT (exact) plus
  Pool/DVE tiles using a Schraudolph bf16 approximation (int16 bitcast);
  softmax normalization cancels its systematic component.
  Diagonal-tile causal mask = multiply with a constant upper-tri bf16
  mask on DVE (4x mode).
  ctx row-major per i-block: ctx[i, h*65+c] accumulates e-chunk^T @
  [v_h | 1] with all 4 heads sharing one PSUM bank (K=1 zero-row opener
  matmul establishes the accumulation group + write ordering).
  Norm: DVE reciprocal of the 4 sums columns + broadcast multiply
  during PSUM eviction -> ctx_n bf16; PE-transpose (bf16) back to
  col-major ectx for the Wo spine.  messages^T = Wo.T @ ectx (+bo).
  u computed ROW-major: u[i,:] = relu(sum_c chunk_c(x^T,m^T).T @ Wu_c
  + bu) with bu folded in via a K=1 opener matmul; direct DMA out, no
  output transposes.
  Extras: PE HAM warm-up + ACT exp-table preload during the input DMA
  window; software-pipelined exp->ctx; projection/tail work interleaved
  into the attention stream to fill PE stall gaps.
"""
import sys

sys.path.insert(0, "/opt/trn_rl_repo")

import numpy as np

import concourse.bass as bass  # noqa: F401
import concourse.mybir as mybir
import concourse.tile as tile
from concourse import bacc
from concourse.masks import make_identity

B, N, DM, H = 8, 2048, 256, 4
HD = DM // H  # 64
NB = N // 128  # 16 i/j blocks
IT = N // 512  # 4 quarters
F32 = mybir.dt.float32
F32R = mybir.dt.float32r
BF16 = mybir.dt.bfloat16
I16 = mybir.dt.int16

# Schraudolph bf16 exp: i16 = trunc(x * SCHR_S + SCHR_B); bitcast -> bf16
SCHR_S = 128.0 / np.log(2.0)
SCHR_B = 127.0 * 128.0 - 5.1
SCALE = 0.125  # 1/sqrt(HD)

import os

# exp-tile engine assignment pattern (A=ACT exact, P=Pool, D=DVE approx)
EXP_PATTERN = os.environ.get("K_EXP_PATTERN", "AAAAAPPD")
PEND_DEPTH = int(os.environ.get("K_PEND_DEPTH", "3"))
WARM_N = int(os.environ.get("K_WARM_N", "12"))
TAIL_D1 = int(os.environ.get("K_TAIL_D1", "1"))
TAIL_D2 = int(os.environ.get("K_TAIL_D2", "2"))
FILL_NUM = int(os.environ.get("K_FILL_NUM", "5"))
FILL_DEN = int(os.environ.get("K_FILL_DEN", "4"))
MASK_DVE_MOD = int(os.environ.get("K_MASK_DVE_MOD", "2"))
import json as _json
EXP_BIAS = _json.loads(os.environ.get("K_EXP_BIAS", '{"A":1.0,"D":1.0,"P":1.0}'))
EXP_SPLIT = int(os.environ.get("K_EXP_SPLIT", "0"))


def build_program():
    nc = bacc.Bacc("TRN2", target_bir_lowering=False, debug=False)
    x_d = nc.dram_tensor("x", [N, DM], F32, kind="ExternalInput").ap()
    wq_d = nc.dram_tensor("wq", [DM, DM], F32, kind="ExternalInput").ap()
    wk_d = nc.dram_tensor("wk", [DM, DM], F32, kind="ExternalInput").ap()
    wv_d = nc.dram_tensor("wv", [DM, DM], F32, kind="ExternalInput").ap()
    wo_d = nc.dram_tensor("wo", [DM, DM], F32, kind="ExternalInput").ap()
    wu_d = nc.dram_tensor("wu", [2 * DM, DM], F32, kind="ExternalInput").ap()
    bq_d = nc.dram_tensor("bq", [DM], F32, kind="ExternalInput").ap()
    bk_d = nc.dram_tensor("bk", [DM], F32, kind="ExternalInput").ap()
    bv_d = nc.dram_tensor("bv", [DM], F32, kind="ExternalInput").ap()
    bo_d = nc.dram_tensor("bo", [DM], F32, kind="ExternalInput").ap()
    bu_d = nc.dram_tensor("bu", [DM], F32, kind="ExternalInput").ap()
    out_d = nc.dram_tensor("out", [N, DM], F32, kind="ExternalOutput").ap()

    def r(ap):
        return ap.bitcast(F32R)

    with tile.TileContext(nc) as tc:
        with (
            tc.tile_pool(name="const", bufs=1) as cpool,
            tc.tile_pool(name="big", bufs=1) as bpool,
            tc.tile_pool(name="work", bufs=3) as wpool,
            tc.tile_pool(name="mm", bufs=1, space="PSUM") as mmp,
            tc.tile_pool(name="sc", bufs=4, space="PSUM") as scp,
            tc.tile_pool(name="ctxp", bufs=2, space="PSUM") as ctxp,
            tc.tile_pool(name="sump", bufs=1, space="PSUM") as sump,
        ):
            # ---- constants / weights (batched DMAs) ----
            ident_f = cpool.tile([128, 128], F32, tag="identf")
            make_identity(nc, ident_f[:])
            ident = cpool.tile([128, 128], F32R, tag="ident")
            nc.vector.tensor_copy(ident[:], ident_f[:])
            ident_b = cpool.tile([128, 128], BF16, tag="identb")
            nc.vector.tensor_copy(ident_b[:], ident_f[:])
            # PE HAM warm-up during the input-DMA window; ACT exp-table
            # preload off the critical path.
            warm = scp.tile([128, 512], F32R, tag="sc", name="warm")
            for _ in range(WARM_N):
                nc.tensor.transpose(warm[0:128, 0:128], ident[:], ident[:])
            wexp = cpool.tile([1, 8], F32, tag="wexp")
            nc.scalar.activation(
                wexp[:], ident_f[0:1, 0:8], mybir.ActivationFunctionType.Exp
            )
            # each W loaded as one DMA: [128, 2*DM], chunk c at cols [c*DM,..)
            wq_a = cpool.tile([128, 2 * DM], F32R, tag="wqa")
            wk_a = cpool.tile([128, 2 * DM], F32R, tag="wka")
            wv_a = cpool.tile([128, 2 * DM], F32R, tag="wva")
            wo_a = cpool.tile([128, 2 * DM], F32R, tag="woa")
            wu_a = cpool.tile([128, 4 * DM], F32R, tag="wua")

            def dma_w(t_sb, t_d):
                nc.sync.dma_start(
                    t_sb[:].rearrange("p (c d) -> p c d", d=DM),
                    r(t_d.rearrange("(c p) d -> p c d", p=128)),
                )

            stage = cpool.tile([128, NB * DM], F32R, tag="stage")
            xs_all = stage
            x_r = r(x_d.rearrange("(t p) d -> p t d", p=128))

            def dma_x(g):
                nc.sync.dma_start(
                    xs_all[:, g * 2 * DM:(g + 1) * 2 * DM].rearrange(
                        "p (t d) -> p t d", d=DM
                    ),
                    x_r[:, g * 2:(g + 1) * 2, :],
                )

            dma_x(0)
            dma_x(1)
            dma_w(wq_a, wq_d)
            dma_w(wk_a, wk_d)
            dma_w(wv_a, wv_d)
            wq_sb = [wq_a[:, c * DM:(c + 1) * DM] for c in range(2)]
            wk_sb = [wk_a[:, c * DM:(c + 1) * DM] for c in range(2)]
            wv_sb = [wv_a[:, c * DM:(c + 1) * DM] for c in range(2)]
            wu_sb = [wu_a[:, c * DM:(c + 1) * DM] for c in range(4)]
            bq_a = cpool.tile([128, 2], F32, tag="bqa")
            bk_a = cpool.tile([128, 2], F32, tag="bka")
            bo_a = cpool.tile([128, 2], F32, tag="boa")
            for t_sb, t_d in ((bq_a, bq_d), (bk_a, bk_d), (bo_a, bo_d)):
                nc.sync.dma_start(t_sb[:], t_d.rearrange("(c p) -> p c", p=128))
            bq_c = [bq_a[:, b:b + 1] for b in range(2)]
            bk_c = [bk_a[:, b:b + 1] for b in range(2)]
            bo_c = [bo_a[:, b:b + 1] for b in range(2)]
            # bu as a row [1, 256] for the K=1 bias-opener matmul
            bu_row = cpool.tile([1, DM], F32, tag="burow")
            nc.sync.dma_start(bu_row[:], bu_d.rearrange("(b a) -> b a", b=1))
            # bv broadcast tile [128, 256] (f32; used by DVE add)
            bv_row = cpool.tile([1, DM], F32, tag="bvrow")
            nc.sync.dma_start(bv_row[:], bv_d.rearrange("(b a) -> b a", b=1))
            ones1 = cpool.tile([1, 128], F32, tag="ones1")
            nc.gpsimd.memset(ones1[:], 1.0)
            bv_bc = cpool.tile([128, DM], F32, tag="bvbc")
            pt0 = mmp.tile([128, DM], F32, tag="mm")
            nc.tensor.matmul(pt0[:], ones1[:], bv_row[:], start=True, stop=True)
            nc.vector.tensor_copy(bv_bc[:], pt0[:])
            zrow = cpool.tile([1, 512], F32, tag="zrow")
            nc.gpsimd.memset(zrow[:], 0.0)
            ones_col4 = cpool.tile([128, 4], BF16, tag="onescol4")
            nc.gpsimd.memset(ones_col4[:], 1.0)
            # constant -240 * strict-lower-triangle bf16 tile: accumulated
            # onto diagonal score chunks so exp() masks them to ~1e-13
            neglt = cpool.tile([128, 128], BF16, tag="neglt")
            nc.gpsimd.memset(neglt[:], -240.0)
            nc.gpsimd.affine_select(
                neglt[:], neglt[:],
                pattern=[[1, 128]],
                compare_op=mybir.AluOpType.is_lt,
                fill=0.0,
                base=0,
                channel_multiplier=-1,
            )
            # ---- remaining weights, then rest of x (x blocks 4+ are not
            # needed until the second quarter's transposes) ----
            dma_w(wo_a, wo_d)
            dma_w(wu_a, wu_d)
            for g in range(2, 8):
                dma_x(g)
            # Wo in bf16 so the m spine can use narrow (128-col) moving
            # tiles; conversion emitted lazily (as a quarter-0 filler) so it
            # doesn't head-block the DVE queue while the Wo DMA lands.
            wo_b = cpool.tile([128, 2 * DM], BF16, tag="wob")
            wo_sb = [wo_b[:, c * DM:(c + 1) * DM] for c in range(2)]

            def unit_wo_conv():
                nc.vector.tensor_copy(wo_b[:], wo_a[:])

            xT = [bpool.tile([128, N], F32R, tag=f"xT{c}", name=f"xT{c}") for c in range(2)]
            qT = [bpool.tile([128, N], BF16, tag=f"qT{b}", name=f"qT{b}") for b in range(2)]
            kT = [bpool.tile([128, N], BF16, tag=f"kT{b}", name=f"kT{b}") for b in range(2)]
            ectx = [bpool.tile([128, N], BF16, tag=f"ectx{b}", name=f"ectx{b}") for b in range(2)]
            m_sb = [bpool.tile([128, N], F32R, tag=f"m{b}", name=f"m{b}") for b in range(2)]
            # v_sb[jb]: [128, 4*65] bf16; head h data at 65h..65h+63, ones at 65h+64
            v_sb = [bpool.tile([128, 4 * 65], BF16, tag=f"v{jb}", name=f"v{jb}") for jb in range(NB)]
            ostage = stage
            out_r = r(out_d.rearrange("(t p) d -> p t d", p=128))

            # ---------- filler units (projection / tail work) ----------
            def unit_xT(ib, alternate=False):
                for c in range(2):
                    tp = scp.tile([128, 128], F32R, tag="sc", name="tp")
                    nc.tensor.transpose(
                        tp[:], xs_all[:, ib * DM + c * 128:ib * DM + (c + 1) * 128],
                        ident_b[:],
                    )
                    if load["D"] + 258 < load["P"] + 273:
                        eng, k, cost = nc.vector, "D", 258
                    else:
                        eng, k, cost = nc.gpsimd, "P", 273
                    load[k] += cost
                    eng.tensor_copy(xT[c][:, ib * 128:(ib + 1) * 128], tp[:])

            def unit_qk(it, which, blk, pool=None):
                w_sb, b_c, dstT = (
                    (wq_sb, bq_c, qT) if which == 0 else (wk_sb, bk_c, kT)
                )
                pool = pool or mmp
                pt = pool.tile([128, 512], F32, tag=pool is mmp and "mm" or "sc", name="qkpt")
                for c in range(2):
                    nc.tensor.matmul(
                        pt[:],
                        w_sb[c][:, blk * 128:(blk + 1) * 128],
                        xT[c][:, it * 512:(it + 1) * 512],
                        start=(c == 0),
                        stop=(c == 1),
                    )
                load["D"] += 658
                nc.vector.tensor_scalar_add(
                    dstT[blk][:, it * 512:(it + 1) * 512], pt[:], b_c[blk][:]
                )

            def unit_v(jb, pool=None):
                v4 = v_sb[jb][:].rearrange("p (h e) -> p h e", e=65)
                nc.vector.tensor_copy(
                    v4[:, :, 64:65],
                    ones_col4[:].rearrange("p (h e) -> p h e", e=1),
                )
                pool = pool or mmp
                pt = pool.tile([128, DM], F32, tag=pool is mmp and "mm" or "sc", name="vpt")
                for c in range(2):
                    nc.tensor.matmul(
                        pt[:],
                        xT[c][:, jb * 128:(jb + 1) * 128],
                        wv_sb[c][:],
                        start=(c == 0),
                        stop=(c == 1),
                    )
                load["D"] += 420
                nc.vector.tensor_tensor(
                    v4[:, :, 0:64],
                    pt[:].rearrange("p (h e) -> p h e", e=64),
                    bv_bc[:].rearrange("p (h e) -> p h e", e=64),
                    op=mybir.AluOpType.add,
                )

            def unit_ctxT(ib, ctxn):
                for c in range(2):
                    tp = scp.tile([128, 128], BF16, tag="sc", name="tpc")
                    nc.tensor.transpose(
                        tp[:], ctxn[:, c * 128:(c + 1) * 128], ident_b[:]
                    )
                    if load["D"] + 258 < load["P"] + 273:
                        eng, k, cost = nc.vector, "D", 258
                    else:
                        eng, k, cost = nc.gpsimd, "P", 273
                    load[k] += cost
                    eng.tensor_copy(ectx[c][:, ib * 128:(ib + 1) * 128], tp[:])

            def unit_m(ib, blk):
                # messages^T for one i-block (128 cols, bf16 moving)
                isl = slice(ib * 128, (ib + 1) * 128)
                pt = mmp.tile([128, 128], F32, tag="mm", name="mpt")
                for c in range(2):
                    nc.tensor.matmul(
                        pt[:],
                        wo_sb[c][:, blk * 128:(blk + 1) * 128],
                        ectx[c][:, isl],
                        start=(c == 0),
                        stop=(c == 1),
                    )
                load["D"] += 258
                nc.vector.tensor_scalar_add(m_sb[blk][:, isl], pt[:], bo_c[blk][:])

            def unit_u(ib):
                up = mmp.tile([128, DM], F32, tag="mm", name="up")
                # K=1 bias opener: writes bu to every row, claims the group
                nc.tensor.matmul(
                    up[:], r(ones1[:]), r(bu_row[:]), start=True, stop=False,
                    skip_group_check=True,
                )
                for c in range(4):
                    lhs = xT[c] if c < 2 else m_sb[c - 2]
                    nc.tensor.matmul(
                        up[:],
                        lhs[:, ib * 128:(ib + 1) * 128],
                        wu_sb[c],
                        start=False,
                        stop=(c == 3),
                        skip_group_check=True,
                    )
                load["D"] += 391
                nc.vector.tensor_scalar_max(
                    ostage[:, ib * DM:(ib + 1) * DM], up[:], 0.0
                )
                nc.sync.dma_start(
                    out_r[:, ib:ib + 1, :],
                    ostage[:, ib * DM:(ib + 1) * DM].rearrange(
                        "p (t d) -> p t d", d=DM
                    ),
                )

            # ---------- attention ----------
            # greedy static load balancing: route each flexible op to the
            # engine with the least accumulated estimated busy time
            load = {"A": 0.0, "D": 0.0, "P": 0.0}

            def emit_exp_span(e, sc_t, lo, hi):
                n = hi - lo
                cost = {
                    "A": 0.833 * n + 215,
                    "D": 1.04 * n + 155,
                    "P": 1.39 * n + 125,
                }
                kind = min(("A", "D", "P"), key=lambda k: load[k] + cost[k] * EXP_BIAS[k])
                load[kind] += cost[kind]
                if kind == "A":
                    nc.scalar.activation(
                        e[:, lo:hi], sc_t[:, lo:hi],
                        mybir.ActivationFunctionType.Exp,
                        scale=SCALE,
                    )
                else:
                    eng = nc.gpsimd if kind == "P" else nc.vector
                    eng.tensor_scalar(
                        e[:, lo:hi].bitcast(I16), sc_t[:, lo:hi],
                        float(SCHR_S * SCALE), float(SCHR_B),
                        op0=mybir.AluOpType.mult, op1=mybir.AluOpType.add,
                    )

            def emit_exp(e, sc_t, skip, w):
                # split at the tile's 256-col boundary: ctx chunks then only
                # wait on the half that covers their 128-col strip
                if EXP_SPLIT and skip < 256 < w:
                    emit_exp_span(e, sc_t, skip, 256)
                    emit_exp_span(e, sc_t, 256, w)
                else:
                    emit_exp_span(e, sc_t, skip, w)

            # ---------- flat attention pipeline across all quarters ----------
            g_i = [0]      # global group counter
            sched = []     # (due_group, fn) deferred tail stages
            pend = []      # scores->ctx software pipeline entries

            def enq(delay, fn):
                sched.append((g_i[0] + delay, fn))

            def run_due():
                for item in list(sched):
                    if item[0] <= g_i[0]:
                        sched.remove(item)
                        item[1]()

            def tail_norm(it, ib, ctx_t, sums):
                ib4 = ib % 4
                rsb = wpool.tile([128, 4], F32, tag="rsb", bufs=2, name="rsb")
                nc.vector.reciprocal(
                    rsb[:].rearrange("p (h e) -> p h e", e=1),
                    sums[:].rearrange("p (h e) -> p h e", e=1)[:, 4 * ib4:4 * ib4 + 4],
                )
                c4 = ctx_t[ib4 // 2][:, 256 * (ib4 % 2):256 * (ib4 % 2) + 256]
                ctxn = wpool.tile([128, DM], BF16, tag="ctxn", bufs=4, name="ctxn")
                in0 = c4.rearrange("p (h e) -> p h e", e=64)
                in1 = rsb[:].rearrange("p (h e) -> p h e", e=1)
                in0b, in1b = bass.broadcast_tensor_aps(in0, in1)
                load["D"] += 530
                nc.vector.tensor_tensor(
                    ctxn[:].rearrange("p (h e) -> p h e", e=64),
                    in0b, in1b, op=mybir.AluOpType.mult,
                )
                enq(TAIL_D2, lambda: tail_ctxT(ib, ctxn))

            def tail_ctxT(ib, ctxn):
                unit_ctxT(ib, ctxn)
                enq(TAIL_D2, lambda: tail_m(ib))

            def tail_m(ib):
                for blk in range(2):
                    unit_m(ib, blk)
                enq(TAIL_D2, lambda: tail_u(ib))

            def tail_u(ib):
                unit_u(ib)

            def emit_ctx(ent):
                jb, h, e, cst, it, st = ent
                ctx_t, sums = st["ctx"], st["sums"]
                if not st["opened"]:
                    # deferred group-openers: must land after the previous
                    # quarter's norm reads of the same PSUM slots
                    st["opened"] = True
                    for i in range(2):
                        nc.tensor.matmul(
                            ctx_t[i][:], r(ones1[:]), r(zrow[:]), start=True,
                            stop=False, skip_group_check=True,
                        )
                    nc.tensor.matmul(
                        sums[:], r(ones1[:]), r(zrow[0:1, 0:16]), start=True,
                        stop=False, skip_group_check=True,
                    )
                for ib in range(max(4 * it, jb), 4 * it + 4):
                    ib4 = ib % 4
                    eoff = 128 * ib4 - (cst - 512 * it)
                    nc.tensor.matmul(
                        ctx_t[ib4 // 2][:, 256 * (ib4 % 2) + 64 * h:
                                        256 * (ib4 % 2) + 64 * h + 64],
                        e[:, eoff:eoff + 128],
                        v_sb[jb][:, 65 * h:65 * h + 64],
                        start=False,
                        stop=(jb == ib),
                        skip_group_check=True,
                    )
                    nc.tensor.matmul(
                        sums[:, 4 * ib4 + h:4 * ib4 + h + 1],
                        e[:, eoff:eoff + 128],
                        v_sb[jb][:, 65 * h + 64:65 * h + 65],
                        start=False,
                        stop=(jb == ib),
                        skip_group_check=True,
                    )
                if jb // 4 == it and h == 3:
                    enq(TAIL_D1, lambda: tail_norm(it, jb, ctx_t, sums))

            def attention_quarter(it, fillers):
                f_i = [0]
                groups = (4 * it + 4) * H
                g_q = [0]

                def fill_pace():
                    # finish the fillers ~80% through the quarter
                    want = (g_q[0] * len(fillers) * FILL_NUM) // (groups * FILL_DEN)
                    while f_i[0] < min(want, len(fillers)):
                        fillers[f_i[0]]()
                        f_i[0] += 1

                # ctx banks: 2 i-blocks per [128, 512] bank; sums in their own
                # bank at col 4*ib4+h; group-openers emitted lazily on first
                # ctx write (see emit_ctx)
                st = {
                    "ctx": [
                        ctxp.tile([128, 512], F32, tag="ctx", name=f"ctx{it}_{i}")
                        for i in range(2)
                    ],
                    "sums": sump.tile([128, 16], F32, tag="sums", name=f"s{it}"),
                    "opened": False,
                }

                for jb in range(4 * it + 4):
                    diag = jb // 4 == it
                    cst = 512 * it + (128 * (jb % 4) if diag else 0)
                    w = 512 * (it + 1) - cst
                    for h in range(H):
                        g_i[0] += 1
                        g_q[0] += 1
                        qh = qT[h // 2][64 * (h % 2):64 * (h % 2) + 64, :]
                        kh = kT[h // 2][64 * (h % 2):64 * (h % 2) + 64, :]
                        sc_t = scp.tile([128, 512], F32, tag="sc", name="sct")
                        nc.tensor.matmul(
                            sc_t[:, 0:w],
                            kh[:, jb * 128:(jb + 1) * 128],
                            qh[:, cst:cst + w],
                            start=True,
                            stop=not diag,
                        )
                        if diag:
                            # causal mask: accumulate -240*strict-LT onto the
                            # diagonal 128-col chunk before exp
                            nc.tensor.matmul(
                                sc_t[:, 0:128], ident_b[:], neglt[:],
                                start=False, stop=True,
                            )
                        e = wpool.tile([128, 512], BF16, tag="e", bufs=8, name="e")
                        emit_exp(e, sc_t, 0, w)
                        pend.append((jb, h, e, cst, it, st))
                        run_due()
                        if len(pend) > PEND_DEPTH:
                            emit_ctx(pend.pop(0))
                        fill_pace()
                while f_i[0] < len(fillers):
                    fillers[f_i[0]]()
                    f_i[0] += 1

            # ---------- main schedule ----------
            # quarter 0 prerequisites up-front
            for ib in range(4):
                unit_xT(ib, alternate=True)
            for which in range(2):
                for blk in range(2):
                    unit_qk(0, which, blk, pool=scp)
            for jb in range(4):
                unit_v(jb, pool=scp)

            for it in range(IT):
                fillers = []
                if it == 0:
                    fillers.append(unit_wo_conv)
                else:
                    # k projections + v for this quarter's new (diagonal)
                    # j-blocks are only needed in the back half of the quarter
                    for blk in range(2):
                        fillers.append(lambda b=blk, t=it: unit_qk(t, 1, b))
                    for jb in range(4 * it, 4 * it + 4):
                        fillers.append(lambda jb=jb: unit_v(jb))
                if it + 1 < IT:
                    for ib in range(4 * (it + 1), 4 * (it + 1) + 4):
                        fillers.append(lambda ib=ib: unit_xT(ib))
                    for blk in range(2):
                        fillers.append(
                            lambda b=blk, t=it + 1: unit_qk(t, 0, b)
                        )
                attention_quarter(it, fillers)

            # drain the pipeline and remaining tail stages
            while pend:
                g_i[0] += 1
                run_due()
                emit_ctx(pend.pop(0))
            while sched:
                g_i[0] += 1
                run_due()

    nc.compile()
    return nc


_STATE = {}


def _get_runner():
    if "run" in _STATE:
        return _STATE["run"]
    import jax
    from concourse.bass2jax import (
        _bass_exec_p,
        install_neuronx_cc_hook,
        partition_id_tensor,
    )
    from jax.sharding import Mesh, PartitionSpec
    from jax.experimental.shard_map import shard_map

    nc = build_program()
    install_neuronx_cc_hook()
    partition_name = nc.partition_id_tensor.name if nc.partition_id_tensor else None
    in_names, out_names, out_avals, zero_outs = [], [], [], []
    for alloc in nc.m.functions[0].allocations:
        if not isinstance(alloc, mybir.MemoryLocationSet):
            continue
        name = alloc.memorylocations[0].name
        if alloc.kind == "ExternalInput":
            if name != partition_name:
                in_names.append(name)
        elif alloc.kind == "ExternalOutput":
            shape = tuple(alloc.tensor_shape)
            dtype = mybir.dt.np(alloc.dtype)
            out_names.append(name)
            out_avals.append(jax.core.ShapedArray(shape, dtype))
            zero_outs.append(np.zeros(shape, dtype))
    n_params = len(in_names)
    all_in = in_names + out_names + ([partition_name] if partition_name else [])

    def _body(*args):
        operands = list(args)
        if partition_name is not None:
            operands.append(partition_id_tensor())
        return tuple(
            _bass_exec_p.bind(
                *operands,
                out_avals=tuple(out_avals),
                in_names=tuple(all_in),
                out_names=tuple(out_names),
                lowering_input_output_aliases=(),
                sim_require_finite=True,
                sim_require_nnan=True,
                nc=nc,
            )
        )

    devices = jax.devices()[:B]
    mesh = Mesh(np.asarray(devices), ("core",))
    specs = (PartitionSpec("core"),) * (n_params + len(out_names))
    jitted = jax.jit(
        shard_map(
            _body, mesh=mesh, in_specs=specs,
            out_specs=(PartitionSpec("core"),) * len(out_names), check_rep=False,
        ),
        keep_unused=True,
    )

    def run(in_maps):
        import jax as _jax

        concat_in = [
            np.concatenate([np.asarray(m[nm]) for m in in_maps], axis=0)
            for nm in in_names
        ]
        concat_zero = [
            np.zeros((B * z.shape[0], *z.shape[1:]), z.dtype) for z in zero_outs
        ]
        outs = jitted(*concat_in, *concat_zero)
        _jax.block_until_ready(outs)
        res = []
        o = np.asarray(outs[out_names.index("out")])
        per = o.shape[0] // B
        for c in range(B):
            res.append(o[c * per:(c + 1) * per])
        return res

    _STATE["run"] = run
    return run


def make_in_maps(node_features, Wq, bq, Wk, bk, Wv, bv, Wo, bo, Wu, bu):
    in_maps = []
    for c in range(B):
        in_maps.append(
            {
                "x": np.ascontiguousarray(node_features[c], dtype=np.float32),
                "wq": np.asarray(Wq, np.float32),
                "wk": np.asarray(Wk, np.float32),
                "wv": np.asarray(Wv, np.float32),
                "wo": np.asarray(Wo, np.float32),
                "wu": np.asarray(Wu, np.float32),
                "bq": np.asarray(bq, np.float32),
                "bk": np.asarray(bk, np.float32),
                "bv": np.asarray(bv, np.float32),
                "bo": np.asarray(bo, np.float32),
                "bu": np.asarray(bu, np.float32),
            }
        )
    return in_maps


def kernel(
    node_features, causal_mask, Wq, bq, Wk, bk, Wv, bv, Wo, bo, Wu, bu
):
    """Full-input entry point: shards batch across 8 cores internally."""
    del causal_mask  # guaranteed tril(ones); mask generated on-chip
    run = _get_runner()
    in_maps = make_in_maps(node_features, Wq, bq, Wk, bk, Wv, bv, Wo, bo, Wu, bu)
    outs = run(in_maps)
    return np.stack(outs, axis=0)
